# revision 45
# baseline (speedup 1.0000x reference)
"""Trainium2 Bass kernel for nn_Block_16544214024520 (dense_cnn).

Data-parallel over batch: 16 samples -> 2 per NeuronCore x 8 cores.
All parameters replicated. Per-sample layout: channels on partitions
(256 = 2 chunks of 128), pixels (64x64 = 4096) on the free dim.

Reference pipeline (per sample):
  gn(32) -> 1x1 conv(256->256)+silu -> gn(16) -> 3x3 grouped conv
  (g=4, 256->512)+silu -> gn(2) -> window-mean(8x8) -> radix amax ->
  1x1 g-conv(256->64)+silu -> gn(8) -> 1x1 g-conv(64->512) ->
  softmax over radix(2) -> gated combine -> channel matmul(256->512?no 256)
  -> gn(32) -> +residual

Optimizations over the straightforward version:
  * conv1 (3x3 grouped) runs 5 matmuls per row-tile instead of 18
    half-width ones: (ky=0,ky=1) taps pair into K=128 matmuls via a
    row-shifted duplicate of the padded input (SBUF->SBUF DMA), and
    (ky=2,kx=0,1) pair via a col-shifted duplicate; one K=64 single
    remains. Each weight block is reused across 4 row-tiles.
  * conv0/conv1/final accumulate into [128,1024] PSUM tiles (2 banks)
    -> half the scalar-engine evacuations.
  * GN3/GN5 statistics come from the scalar engine: means accumulate
    for free on the evacuation (accum_out), E[x^2] via a Square pass,
    replacing ~70us of DVE bn_stats.
  * radix softmax gating is a pure bf16 multiply: since a0+a1==1, the
    -mean3 shift const-folds into the final-evacuation bias.
  * window-pool partials in one tensor_reduce per group; per-group
    conv3->sigmoid->gate pipeline so gating starts early.
  * residual is loaded just-in-time in [128,1024] quarters; GN5 apply
    and the residual add alternate between gpsimd and DVE.
  * two samples emitted with a skewed, split-conv1 schedule so the
    second sample's conv1 fills the first sample's attention phase.
"""

import os
import sys

for _p in ("/opt/trn_rl_repo", "/opt/pypackages"):
    if _p not in sys.path:
        sys.path.append(_p)

import ml_dtypes
import numpy as np

import concourse.bass as bass  # noqa: F401
import concourse.mybir as mybir
import concourse.tile as tile
from concourse import bacc
from concourse.masks import make_identity

F32 = mybir.dt.float32
BF16 = mybir.dt.bfloat16
AF = mybir.ActivationFunctionType
ALU = mybir.AluOpType
AX = mybir.AxisListType

NCORES = 8
BPC = 2          # samples per core
C = 256          # channels
H = W = 64
NPIX = H * W     # 4096
PADW = W + 2     # 66
Hn = Wn = 8      # window grid
WS = 8           # window size
EPS = 1e-5
NT = 8           # n-tiles of 512 pixels (8 rows of 64)
XGROWS = 65      # rows in the shifted-dup conv1 input buffer


# ---------------------------------------------------------------- host prep

def _host_consts():
    """Constant matrices shared by all cores (built once)."""
    c = {}
    # GN over 256 channels, 32 groups of 8 (GN1/GN5)
    gm1 = np.zeros((2, 128, 32), np.float32)
    rep1 = np.zeros((2, 128, 128), np.float32)
    for ch in range(2):
        for k in range(128):
            g = (128 * ch + k) // 8
            gm1[ch, k, g] = 1.0 / 8.0
        for m in range(128):
            rep1[ch, (128 * ch + m) // 8 % 128, m] = 1.0
    c["gm1"] = gm1
    c["rep1"] = rep1
    # GN2: 16 groups of 16 over 256 channels
    gm2 = np.zeros((2, 128, 16), np.float32)
    rep2 = np.zeros((2, 128, 128), np.float32)
    for ch in range(2):
        for k in range(128):
            gm2[ch, k, (128 * ch + k) // 16] = 1.0 / 16.0
        for m in range(128):
            rep2[ch, (128 * ch + m) // 16, m] = 1.0
    c["gm2"] = gm2
    c["rep2"] = rep2
    # GN3 over 512 channels, 2 groups of 256 (chunks 0,1 -> g0; 2,3 -> g1)
    g3 = np.zeros((4, 128, 2), np.float32)
    r3 = np.zeros((4, 128, 128), np.float32)
    for mc in range(4):
        g3[mc, :, mc // 2] = 1.0 / 256.0
        r3[mc, mc // 2, :] = 1.0
    c["g3"] = g3
    c["r3"] = r3
    # GN4 over 64 channels, 8 groups of 8
    g4 = np.zeros((128, 8), np.float32)
    for k in range(64):
        g4[k, k // 8] = 1.0 / 8.0
    r4 = np.zeros((128, 64), np.float32)
    for m in range(64):
        r4[m // 8, m] = 1.0
    c["g4"] = g4
    c["r4"] = r4
    return c


def _host_weights(w0, b0, w1, b1, w2, b2, w3, b3, weight):
    """Rearrange torch-layout conv weights into matmul lhsT tensors."""
    d = {}
    # conv0: out[o,p] = sum_i w0[o,i] x[i,p]  -> lhsT[i,o]
    d["w0T"] = np.ascontiguousarray(w0[:, :, 0, 0].T).astype(
        ml_dtypes.bfloat16)  # [256,256]
    d["b0c"] = np.ascontiguousarray(b0.reshape(C, 1)).astype(np.float32)
    # conv1: grouped 3x3, groups=4 (in 64 -> out 128 each).
    # Row-pair lhsT per (g, dx): [128,128] rows 0:64 = ky=0, rows 64:128 =
    # ky=1 (paired via the row-shifted dup buffer xg).
    # Col-pair lhsT per g: rows 0:64 = (ky=2, kx=0), rows 64:128 =
    # (ky=2, kx=1) (paired via the col-shifted dup buffer xh).
    # Last single per g: rows 0:64 = (ky=2, kx=2).
    w1p = np.zeros((4, 3, 128, 128), np.float32)
    w1c = np.zeros((4, 128, 128), np.float32)
    w1e = np.zeros((4, 128, 128), np.float32)
    for g in range(4):
        for dx in range(3):
            w1p[g, dx, 0:64, :] = w1[g * 128:(g + 1) * 128, :, 0, dx].T
            w1p[g, dx, 64:128, :] = w1[g * 128:(g + 1) * 128, :, 1, dx].T
        w1c[g, 0:64, :] = w1[g * 128:(g + 1) * 128, :, 2, 0].T
        w1c[g, 64:128, :] = w1[g * 128:(g + 1) * 128, :, 2, 1].T
        w1e[g, 0:64, :] = w1[g * 128:(g + 1) * 128, :, 2, 2].T
    d["w1p"] = w1p.astype(ml_dtypes.bfloat16)
    d["w1c"] = w1c.astype(ml_dtypes.bfloat16)
    d["w1e"] = w1e.astype(ml_dtypes.bfloat16)
    d["b1c"] = np.ascontiguousarray(b1.reshape(2 * C, 1)).astype(np.float32)
    # conv2: groups=2 (in 128 -> out 32)
    w2t = np.zeros((2, 128, 32), np.float32)
    for g in range(2):
        w2t[g] = w2[g * 32:(g + 1) * 32, :, 0, 0].T
    d["w2t"] = w2t
    d["b2c"] = np.ascontiguousarray(b2.reshape(64, 1)).astype(np.float32)
    # conv3: groups=2 (in 32 -> out 256); K padded to 128 with zero rows.
    w3t = np.zeros((4, 128, 128), np.float32)
    for g in range(4):
        src = w3[g * 128:(g + 1) * 128, :, 0, 0]      # [128, 32]
        r0 = 0 if g < 2 else 32
        w3t[g, r0:r0 + 32, :] = src.T
    d["w3t"] = w3t
    # final einsum: out[c,p] = sum_C weight[C,c] z[C,p], z[C] = zint[2C]+zint[2C+1]
    # fold the radix pair-sum by duplicating rows: wdup[c512, c] = weight[c512//2, c]
    wdup = np.repeat(weight.astype(np.float32), 2, axis=0)   # [512, 256]
    d["wdupT"] = np.ascontiguousarray(wdup).astype(ml_dtypes.bfloat16)
    return d


def _pack_consts(wd, cm):
    """Pack all fp32 constants into one [128, F] tensor and all bf16
    weights into another, so startup needs only two DMAs."""
    fcols = []   # list of [128, n] fp32 blocks
    def addf(x):
        x = np.asarray(x, np.float32)
        assert x.shape[0] == 128
        fcols.append(x.reshape(128, -1))
    for c in range(2):
        addf(cm["gm1"][c]); addf(cm["rep1"][c])
        addf(cm["gm2"][c]); addf(cm["rep2"][c])
    for g in range(4):
        addf(cm["g3"][g]); addf(cm["r3"][g])
    addf(cm["g4"]); addf(cm["r4"])
    b0 = wd["b0c"].reshape(2, 128, 1)
    addf(b0[0]); addf(b0[1])
    b1 = wd["b1c"].reshape(4, 128, 1)
    for g in range(4):
        addf(b1[g])
    b2p = np.zeros((128, 1), np.float32)
    b2p[0:64] = wd["b2c"]
    addf(b2p)
    addf(np.full((128, 1), EPS, np.float32))
    for g in range(2):
        addf(wd["w2t"][g])
    for g in range(4):
        addf(wd["w3t"][g])
    cpack = np.concatenate(fcols, axis=1)

    w0 = np.asarray(wd["w0T"])
    bcols = [w0[0:128], w0[128:256]]
    for g in range(4):
        for dx in range(3):
            bcols.append(np.asarray(wd["w1p"])[g, dx])
        bcols.append(np.asarray(wd["w1c"])[g])
        bcols.append(np.asarray(wd["w1e"])[g])
    wdp = np.asarray(wd["wdupT"])
    for k in range(4):
        bcols.append(wdp[k * 128:(k + 1) * 128])
    bpack = np.concatenate(bcols, axis=1).astype(ml_dtypes.bfloat16)
    return cpack, bpack


NCF = 32 + 128 + 16 + 128 + 32 + 128 + 16 + 128 + 4 * (2 + 128) \
    + 8 + 64 + 2 + 4 + 1 + 1 + 2 * 32 + 4 * 128
NBF = 256 * 2 + 4 * 5 * 128 + 4 * 256


# ---------------------------------------------------------------- builder

def build_nc(sim_safe: bool = False):
    nc = bacc.Bacc("TRN2", target_bir_lowering=False, debug=False,
                   num_devices=NCORES)

    def din(name, shape, dt=F32):
        return nc.dram_tensor(name, list(shape), dt, kind="ExternalInput").ap()

    hs = din("hs", (BPC, C, H, W))
    hsb = din("hsb", (BPC, C, H, W), BF16)
    cpack_d = din("cpack", (128, NCF))
    bpack_d = din("bpack", (128, NBF), BF16)

    out_d = nc.dram_tensor("out", [BPC, C, H, W], F32, kind="ExternalOutput").ap()

    with tile.TileContext(nc) as tc:
        with tc.tile_pool(name="consts", bufs=1) as cst, \
             tc.tile_pool(name="b16", bufs=8) as pb16, \
             tc.tile_pool(name="xq", bufs=2) as pxq, \
             tc.tile_pool(name="xg", bufs=2) as pxg, \
             tc.tile_pool(name="xh", bufs=2) as pxh, \
             tc.tile_pool(name="f32", bufs=2) as pf32, \
             tc.tile_pool(name="xr", bufs=3) as pxr, \
             tc.tile_pool(name="small", bufs=2) as sm, \
             tc.tile_pool(name="psum", bufs=2, space="PSUM") as psp:

            # ---- load constants / weights (two packed DMAs) ----
            cpk = cst.tile([128, NCF], F32, name="cpk")
            nc.sync.dma_start(out=cpk, in_=cpack_d)
            bpk = cst.tile([128, NBF], BF16, name="bpk")
            nc.sync.dma_start(out=bpk, in_=bpack_d)

            class _Cur:
                def __init__(self):
                    self.o = 0
            _cf, _cb = _Cur(), _Cur()

            def fsl(n):
                s = cpk[:, _cf.o:_cf.o + n]
                _cf.o += n
                return s

            def bsl(n):
                s = bpk[:, _cb.o:_cb.o + n]
                _cb.o += n
                return s

            gm1_t, rep1_t, gm2_t, rep2_t = [], [], [], []
            for c in range(2):
                gm1_t.append(fsl(32)); rep1_t.append(fsl(128))
                gm2_t.append(fsl(16)); rep2_t.append(fsl(128))
            g3_t, r3_t = [], []
            for g in range(4):
                g3_t.append(fsl(2)); r3_t.append(fsl(128))
            g4_t = fsl(8); r4_t = fsl(64)
            b0_t = [fsl(1) for _ in range(2)]
            b1_t = [fsl(1) for _ in range(4)]
            b2_t = fsl(1)
            eps_t = fsl(1)
            w2_t = [fsl(32) for _ in range(2)]
            w3_t = [fsl(128) for _ in range(4)]
            assert _cf.o == NCF
            w0_t = [bsl(256) for _ in range(2)]
            w1p_t = [[None] * 3 for _ in range(4)]
            w1c_t = [None] * 4
            w1e_t = [None] * 4
            for g in range(4):
                for dx in range(3):
                    w1p_t[g][dx] = bsl(128)
                w1c_t[g] = bsl(128)
                w1e_t[g] = bsl(128)
            wd_t = [bsl(256) for _ in range(4)]
            assert _cb.o == NBF
            ident = cst.tile([128, 128], F32, name="ident")
            make_identity(nc, ident)

            # ------------------------------------------------ helpers
            def silu_evac(out_ap, psum_ap, bias_ap, tag, accum_out=None):
                """out = silu(psum + bias); fused on HW, 2-op in CoreSim."""
                if not sim_safe:
                    nc.scalar.activation(out=out_ap, in_=psum_ap, func=AF.Silu,
                                         bias=bias_ap, scale=1.0,
                                         accum_out=accum_out)
                    return
                if True:
                    ff = psum_ap.free_size()
                    pp = psum_ap.partition_size()
                    sgf = sm.tile([128, 1024], F32, tag="sg", bufs=1,
                                  name=f"sg_{tag}", uniquify=True)
                    sgt = sgf[0:pp, 0:ff]
                    nc.scalar.activation(out=sgt, in_=psum_ap, func=AF.Sigmoid,
                                         bias=bias_ap, scale=1.0)
                    nc.vector.scalar_tensor_tensor(
                        out=out_ap, in0=psum_ap, scalar=bias_ap, in1=sgt,
                        op0=ALU.add, op1=ALU.mult)
                    if accum_out is not None:
                        nc.scalar.activation(out=sgt, in_=out_ap,
                                             func=AF.Identity, scale=1.0,
                                             accum_out=accum_out)

            def gn_scale_bias(mvs, gmat_list, rmat_list, ngroups, tag,
                              ncols=2, raw_ex2=False):
                """Per-channel (scale, bias) tiles for a group norm.

                mvs: list of [128, 2] SBUF tiles of per-channel (mean, var).
                Returns list of [128, ncols] tiles (col0 = rstd,
                col1 = -mean*rstd, col2 = -mean) replicated back to channels.
                """
                nchunk = len(mvs)
                if raw_ex2:
                    # mvs are already [128, 2] = (mean, E[x^2]) tiles
                    rstats = mvs
                else:
                    rstats = []
                    for ci, mv in enumerate(mvs):
                        r = sm.tile([128, 2], F32, tag=f"r_{tag}",
                                    bufs=2 * nchunk)
                        nc.vector.tensor_copy(out=r[:, 0:1], in_=mv[:, 0:1])
                        nc.vector.scalar_tensor_tensor(
                            out=r[:, 1:2], in0=mv[:, 0:1], scalar=mv[:, 0:1],
                            in1=mv[:, 1:2], op0=ALU.mult, op1=ALU.add)
                        rstats.append(r)
                pg = psp.tile([128, 2], F32, tag="gn_ps", bufs=1)
                for ci in range(nchunk):
                    nc.tensor.matmul(pg[0:ngroups, :], gmat_list[ci], rstats[ci],
                                     start=(ci == 0), stop=(ci == nchunk - 1))
                gt = sm.tile([128, 2], F32, tag=f"gt_{tag}", bufs=2)
                nc.vector.memset(gt, 0.0)
                nc.scalar.copy(out=gt[0:ngroups, :], in_=pg[0:ngroups, :])
                # -var = mean^2 - E[x^2]
                negv = sm.tile([128, 1], F32, tag=f"nv_{tag}", bufs=2)
                nc.vector.scalar_tensor_tensor(
                    out=negv[0:ngroups], in0=gt[0:ngroups, 0:1],
                    scalar=gt[0:ngroups, 0:1], in1=gt[0:ngroups, 1:2],
                    op0=ALU.mult, op1=ALU.subtract)
                sd = sm.tile([128, 1], F32, tag=f"sd_{tag}", bufs=2)
                nc.scalar.activation(out=sd[0:ngroups], in_=negv[0:ngroups],
                                     func=AF.Sqrt, bias=eps_t[0:ngroups],
                                     scale=-1.0)
                rstd = sm.tile([128, 1], F32, tag=f"rs_{tag}", bufs=2)
                nc.vector.reciprocal(out=rstd[0:ngroups], in_=sd[0:ngroups])
                stg = sm.tile([128, 3], F32, tag=f"st_{tag}", bufs=2)
                nc.vector.memset(stg, 0.0)
                nc.vector.tensor_copy(out=stg[0:ngroups, 0:1], in_=rstd[0:ngroups])
                nc.vector.tensor_scalar(
                    out=stg[0:ngroups, 1:2], in0=gt[0:ngroups, 0:1],
                    scalar1=rstd[0:ngroups], scalar2=-1.0,
                    op0=ALU.mult, op1=ALU.mult)
                if ncols == 3:
                    nc.vector.tensor_scalar(
                        out=stg[0:ngroups, 2:3], in0=gt[0:ngroups, 0:1],
                        scalar1=-1.0, scalar2=None, op0=ALU.mult)
                scs = []
                for ci, rmat in enumerate(rmat_list):
                    mm = rmat.shape[-1]
                    pr = psp.tile([128, 3], F32, tag="gn_ps", bufs=1)
                    nc.tensor.matmul(pr[0:mm, 0:ncols], rmat,
                                     stg[:, 0:ncols], start=True, stop=True)
                    sc = sm.tile([128, 3], F32, tag=f"sc_{tag}",
                                 bufs=2 * nchunk)
                    nc.scalar.copy(out=sc[0:mm, 0:ncols], in_=pr[0:mm, 0:ncols])
                    scs.append(sc)
                return scs

            st = [dict() for _ in range(BPC)]

            # ------------------------------------------------ phases
            def ph_load(b):
                """Load input, GN1 stats, fold GN1 into conv0 weights."""
                S = st[b]
                hsbv = hsb[b].rearrange("c h w -> c (h w)")
                S["xw"] = [pb16.tile([128, NPIX], BF16, tag="b16",
                                     name=f"xw{b}_{i}") for i in range(2)]
                bst1 = [sm.tile([128, NT, 6], F32, tag="bst1", bufs=2,
                                name=f"bst1_{b}_{i}") for i in range(2)]
                for c in range(2):
                    nc.sync.dma_start(out=S["xw"][c],
                                      in_=hsbv[c * 128:(c + 1) * 128, :])
                    for n in range(NT):
                        nc.vector.bn_stats(out=bst1[c][:, n, :],
                                           in_=S["xw"][c][:, bass.ts(n, 512)])
                mv1 = []
                for c in range(2):
                    mv = sm.tile([128, 2], F32, tag="mv1", bufs=2,
                                 name=f"mv1_{b}_{c}")
                    nc.vector.bn_aggr(out=mv, in_=bst1[c])
                    mv1.append(mv)
                sc1 = gn_scale_bias(mv1, gm1_t, rep1_t, 32, "gn1")
                # fold GN1 into conv0 weights
                w0s = [sm.tile([128, 256], BF16, tag="w0s", bufs=2,
                               name=f"w0s{b}_{i}") for i in range(2)]
                t1b = [sm.tile([128, 1], BF16, tag="t1b", bufs=2,
                               name=f"t1b{b}_{i}") for i in range(2)]
                for c in range(2):
                    nc.vector.tensor_scalar_mul(out=w0s[c], in0=w0_t[c],
                                                scalar1=sc1[c][:, 0:1])
                    nc.vector.tensor_copy(out=t1b[c], in_=sc1[c][:, 1:2])
                b0p = [sm.tile([128, 1], F32, tag="b0p", bufs=2,
                               name=f"b0p{b}_{i}") for i in range(2)]
                for m in range(2):
                    pb = psp.tile([128, 1], F32, tag="gn_ps", bufs=1)
                    for kc in range(2):
                        nc.tensor.matmul(
                            pb,
                            w0s[kc][:, m * 128:(m + 1) * 128],
                            t1b[kc],
                            start=(kc == 0), stop=(kc == 1))
                    nc.scalar.activation(out=b0p[m], in_=pb,
                                         func=AF.Identity, bias=b0_t[m],
                                         scale=1.0)
                S["w0s"] = w0s
                S["b0p"] = b0p

            def ph_conv0(b):
                """conv0 (1x1)+silu straight into padded conv1 input; GN2
                stats + in-place apply; build shifted-dup buffers by DMA."""
                S = st[b]
                xq = [pxq.tile([128, PADW, PADW], BF16, tag="xq",
                               name=f"xq{b}_{i}") for i in range(2)]
                S["xq"] = xq
                for c in range(2):
                    xpf = xq[c]
                    nc.gpsimd.memset(xpf[:, 0:1, :], 0.0)
                    nc.gpsimd.memset(xpf[:, PADW - 1:PADW, :], 0.0)
                    nc.gpsimd.memset(xpf[:, 1:PADW - 1, 0:1], 0.0)
                    nc.gpsimd.memset(xpf[:, 1:PADW - 1, PADW - 1:PADW], 0.0)
                y0 = [pb16.tile([128, NPIX], BF16, tag="b16",
                                name=f"y0{b}_{i}") for i in range(2)]
                bst2 = [sm.tile([128, NT, 6], F32, tag="bst2", bufs=2,
                                name=f"bst2_{b}_{i}") for i in range(2)]
                for m in range(2):
                    for nq in range(4):
                        pacc = psp.tile([128, 1024], F32, tag="acc", bufs=3,
                                        name=f"pc0_{b}_{m}_{nq}",
                                        uniquify=True)
                        for ni in range(2):
                            n = nq * 2 + ni
                            for kc in range(2):
                                nc.tensor.matmul(
                                    pacc[:, ni * 512:(ni + 1) * 512],
                                    S["w0s"][kc][:, m * 128:(m + 1) * 128],
                                    S["xw"][kc][:, bass.ts(n, 512)],
                                    start=(kc == 0), stop=(kc == 1))
                        nsl = bass.ts(nq, 1024)
                        silu_evac(y0[m][:, nsl], pacc, S["b0p"][m],
                                  f"c0_{b}")
                        for ni in range(2):
                            n = nq * 2 + ni
                            nc.vector.bn_stats(out=bst2[m][:, n, :],
                                               in_=y0[m][:, bass.ts(n, 512)])
                mv2 = []
                for c in range(2):
                    mv = sm.tile([128, 2], F32, tag="mv2", bufs=2,
                                 name=f"mv2_{b}_{c}")
                    nc.vector.bn_aggr(out=mv, in_=bst2[c])
                    mv2.append(mv)
                sc2 = gn_scale_bias(mv2, gm2_t, rep2_t, 16, "gn2")
                for c in range(2):
                    nc.gpsimd.tensor_scalar(
                        out=xq[c][:, 1:65, 1:65],
                        in0=y0[c].rearrange("p (h w) -> p h w", h=H),
                        scalar1=sc2[c][:, 0:1], scalar2=sc2[c][:, 1:2],
                        op0=ALU.mult, op1=ALU.add)
                # shifted-dup buffers for conv1 tap pairing:
                # parts 0:64 <- xq rows 0..64 (offset r*66 holds image row r-1)
                # parts 64:128 <- xq rows 1..65 (offset r*66 holds image row r)
                xg = [pxg.tile([128, XGROWS, PADW], BF16, tag="xg",
                               name=f"xg{b}_{g}") for g in range(4)]
                xh = [pxh.tile([128, XGROWS, PADW], BF16, tag="xh",
                               name=f"xh{b}_{g}") for g in range(4)]
                S["xg"] = xg
                S["xh"] = xh
                for g in range(4):
                    kc, blk = g // 2, g % 2
                    src = xq[kc]
                    p0 = blk * 64
                    nc.sync.dma_start(
                        out=xg[g][0:64, :, :],
                        in_=src[p0:p0 + 64, 0:XGROWS, :])
                    nc.sync.dma_start(
                        out=xg[g][64:128, :, :],
                        in_=src[p0:p0 + 64, 1:1 + XGROWS, :])
                    nc.sync.dma_start(
                        out=xh[g][0:64, :, :],
                        in_=src[p0:p0 + 64, 1:1 + XGROWS, :])
                    nc.sync.dma_start(
                        out=xh[g][64:128, :, 0:PADW - 1],
                        in_=src[p0:p0 + 64, 1:1 + XGROWS, 1:PADW])

            def ph_conv1(b, gs):
                """conv1 (3x3 grouped, tap-paired) + silu -> y1 for groups
                in gs; GN3 stats and window-pool partials in-loop."""
                S = st[b]
                if 0 in gs:
                    S["y1"] = [pb16.tile([128, NPIX], BF16, tag="b16",
                                         name=f"y1{b}_{g}") for g in range(4)]
                    S["s13"] = [sm.tile([128, 4], F32, tag="s13", bufs=4,
                                        name=f"s13_{b}_{g}") for g in range(4)]
                    S["s23"] = [sm.tile([128, 2], F32, tag="s23", bufs=4,
                                        name=f"s23_{b}_{g}") for g in range(4)]
                    S["amT"] = sm.tile([64, 256], F32, tag="amT", bufs=1,
                                       name=f"amT{b}")
                y1, s13, s23, amT = S["y1"], S["s13"], S["s23"], S["amT"]
                for g in gs:
                    pa_g = sm.tile([128, NT * 64], BF16, tag="pa", bufs=2,
                                   name=f"pa{b}_{g}")
                    xgv = S["xg"][g]
                    xhv = S["xh"][g]
                    for np2 in range(2):
                        paccs = [psp.tile([128, 1024], F32, tag="acc", bufs=3,
                                          name=f"pc1_{b}_{g}_{np2}_{t}",
                                          uniquify=True) for t in range(2)]
                        # each weight block is loaded once and streamed over
                        # 4 row-tiles (2 psum tiles x 2 halves)
                        for dx in range(3):
                            for t in range(2):
                                for ni in range(2):
                                    n = (np2 * 2 + t) * 2 + ni
                                    r0 = n * WS
                                    nc.tensor.matmul(
                                        paccs[t][:, ni * 512:(ni + 1) * 512],
                                        w1p_t[g][dx],
                                        xgv[:, r0:r0 + 8, dx:dx + 64],
                                        start=(dx == 0), stop=False)
                        for t in range(2):
                            for ni in range(2):
                                n = (np2 * 2 + t) * 2 + ni
                                r0 = n * WS
                                nc.tensor.matmul(
                                    paccs[t][:, ni * 512:(ni + 1) * 512],
                                    w1c_t[g],
                                    xhv[:, r0 + 1:r0 + 9, 0:64],
                                    start=False, stop=False)
                        for t in range(2):
                            for ni in range(2):
                                n = (np2 * 2 + t) * 2 + ni
                                r0 = n * WS
                                nc.tensor.matmul(
                                    paccs[t][:, ni * 512:(ni + 1) * 512],
                                    w1e_t[g][0:64, :],
                                    xhv[0:64, r0 + 1:r0 + 9, 2:66],
                                    start=False, stop=True)
                        for t in range(2):
                            npair = np2 * 2 + t
                            nsl = bass.ts(npair, 1024)
                            silu_evac(y1[g][:, nsl], paccs[t], b1_t[g],
                                      f"c1_{b}",
                                      accum_out=s13[g][:, npair:npair + 1])
                        sqd = sm.tile([128, 2048], BF16, tag="sqd",
                                      bufs=1, name=f"sqd3_{b}_{g}_{np2}",
                                      uniquify=True)
                        nc.scalar.activation(
                            out=sqd, in_=y1[g][:, bass.ts(np2, 2048)],
                            func=AF.Square, scale=1.0,
                            accum_out=s23[g][:, np2:np2 + 1])
                    # window-pool partials in one reduce per group
                    with nc.allow_low_precision(reason="bf16 pool partials"):
                        nc.vector.tensor_reduce(
                            out=pa_g,
                            in_=y1[g].rearrange("p (a w2) -> p a w2", w2=WS),
                            axis=AX.X, op=ALU.add)
                    # finish this group's window means + transpose + radix max
                    pooled = sm.tile([128, Hn, Wn], F32, tag="pooled", bufs=2,
                                     name=f"pooled{b}_{g}", uniquify=True)
                    pav = pa_g.rearrange("p (hn h2 wn) -> p hn wn h2",
                                         hn=Hn, h2=WS)
                    nc.vector.tensor_reduce(out=pooled, in_=pav,
                                            axis=AX.X, op=ALU.add)
                    ptp = psp.tile([64, 128], F32, tag="tp", bufs=1)
                    nc.tensor.transpose(
                        ptp, pooled.rearrange("p a b -> p (a b)"), ident)
                    pooledT = sm.tile([64, 128], F32, tag="pooledT", bufs=2,
                                      name=f"pooledT{b}_{g}", uniquify=True)
                    nc.scalar.copy(out=pooledT, in_=ptp)
                    pv = pooledT.rearrange("p (a b) -> p a b", b=2)
                    nc.vector.tensor_tensor(
                        out=amT[:, g * 64:(g + 1) * 64],
                        in0=pv[:, :, 0], in1=pv[:, :, 1], op=ALU.max)
                if 3 not in gs:
                    return
                mv3 = []
                for g in range(4):
                    mv = sm.tile([128, 2], F32, tag="mv3", bufs=4,
                                 name=f"mv3_{b}_{g}")
                    nc.vector.tensor_reduce(out=mv[:, 0:1], in_=s13[g],
                                            axis=AX.X, op=ALU.add)
                    nc.vector.tensor_reduce(out=mv[:, 1:2], in_=s23[g],
                                            axis=AX.X, op=ALU.add)
                    nc.vector.tensor_scalar(
                        out=mv, in0=mv, scalar1=1.0 / NPIX, scalar2=None,
                        op0=ALU.mult)
                    mv3.append(mv)
                sc3 = gn_scale_bias(mv3, g3_t, r3_t, 2, "gn3", ncols=3,
                                    raw_ex2=True)
                S["sc3"] = sc3
                # fold GN3 scale into the final matmul weights
                wds = [sm.tile([128, 256], BF16, tag="wds", bufs=4,
                               name=f"wds{b}_{kc}") for kc in range(4)]
                t3b = [sm.tile([128, 1], BF16, tag="t3b", bufs=4,
                               name=f"t3b{b}_{kc}") for kc in range(4)]
                for kc in range(4):
                    nc.vector.tensor_scalar_mul(
                        out=wds[kc], in0=wd_t[kc],
                        scalar1=sc3[kc][:, 0:1])
                    # 0.5: the wdup row-duplication would count m3 twice
                    nc.vector.tensor_scalar(
                        out=t3b[kc], in0=sc3[kc][:, 2:3], scalar1=0.5,
                        scalar2=None, op0=ALU.mult)
                # const-fold: cm[co] = sum_C wds[C,co] * (-m3[C]); becomes
                # the final-evac bias (valid because a0 + a1 == 1).
                cmt = [sm.tile([128, 1], F32, tag="cmt", bufs=2,
                               name=f"cmt{b}_{m}") for m in range(2)]
                for m in range(2):
                    pcm = psp.tile([128, 1], F32, tag="gn_ps", bufs=1)
                    for kc in range(4):
                        nc.tensor.matmul(
                            pcm, wds[kc][:, m * 128:(m + 1) * 128], t3b[kc],
                            start=(kc == 0), stop=(kc == 3))
                    nc.scalar.copy(out=cmt[m], in_=pcm)
                S["cmt"] = cmt
                S["wds"] = wds

            def ph_attn(b):
                """Window mean finish, radix amax, conv2+GN4+conv3,
                softmax -> per-group gate tiles; also load the residual."""
                S = st[b]
                sc3 = S["sc3"]
                amT = S["amT"]
                am = [sm.tile([128, 64], F32, tag="am", bufs=2,
                              name=f"am{b}_{i}") for i in range(2)]
                s64 = [sm.tile([128, 1], F32, tag="s64", bufs=2,
                               name=f"s64_{b}_{i}") for i in range(2)]
                for c in range(2):
                    pta = psp.tile([128, 64], F32, tag="tp", bufs=1)
                    nc.tensor.transpose(pta, amT[:, c * 128:(c + 1) * 128],
                                        ident[0:64, 0:64])
                    nc.scalar.copy(out=am[c], in_=pta)
                    # normalize the pooled maxima: am = am*(s3/64) + t3
                    nc.vector.tensor_scalar(
                        out=s64[c], in0=sc3[2 * c][:, 0:1],
                        scalar1=1.0 / (WS * WS), scalar2=None, op0=ALU.mult)
                    nc.vector.tensor_scalar(
                        out=am[c], in0=am[c], scalar1=s64[c],
                        scalar2=sc3[2 * c][:, 1:2], op0=ALU.mult, op1=ALU.add)

                # ---- conv2 (1x1 g=2, 256->64) + silu ----
                p2 = psp.tile([128, 64], F32, tag="tp", bufs=1)
                for g in range(2):
                    nc.tensor.matmul(p2[g * 32:(g + 1) * 32, :], w2_t[g], am[g],
                                     start=True, stop=True)
                a2 = sm.tile([128, 64], F32, tag="a2", bufs=2)
                nc.vector.memset(a2, 0.0)
                silu_evac(a2[0:64, :], p2[0:64, :], b2_t[0:64], f"c2_{b}")

                # ---- GN4 -> a2n ----
                mv4pad = sm.tile([128, 2], F32, tag="mv4", bufs=2)
                nc.vector.memset(mv4pad, 0.0)
                bst4 = sm.tile([128, 1, 6], F32, tag="bst4", bufs=2)
                nc.vector.bn_stats(out=bst4[0:64], in_=a2[0:64].unsqueeze(1))
                nc.vector.bn_aggr(out=mv4pad[0:64], in_=bst4[0:64])
                sc4 = gn_scale_bias([mv4pad], [g4_t], [r4_t], 8, "gn4")[0]
                a2n = sm.tile([128, 64], F32, tag="a2n", bufs=2)
                nc.vector.memset(a2n, 0.0)
                nc.vector.tensor_scalar(
                    out=a2n[0:64], in0=a2[0:64],
                    scalar1=sc4[0:64, 0:1], scalar2=sc4[0:64, 1:2],
                    op0=ALU.mult, op1=ALU.add)

                # ---- conv3 (1x1 g=2, 64->512), b3 = 0; then softmax over
                # radix == sigmoid of pair difference; fully per-group so the
                # first gate tile is ready early ----
                for g in range(4):
                    p3 = psp.tile([128, 64], F32, tag="tp", bufs=1)
                    nc.tensor.matmul(p3, w3_t[g], a2n, start=True, stop=True)
                    a3 = sm.tile([128, 64], F32, tag="a3", bufs=2)
                    nc.scalar.copy(out=a3, in_=p3)
                    p3t = psp.tile([64, 128], F32, tag="tp", bufs=1)
                    nc.tensor.transpose(p3t, a3, ident)
                    a3Tg = sm.tile([64, 128], F32, tag="a3T", bufs=2,
                                   name=f"a3T{b}_{g}", uniquify=True)
                    nc.scalar.copy(out=a3Tg, in_=p3t)
                    a3v = a3Tg.rearrange("p (a b) -> p a b", b=2)
                    dTg = sm.tile([64, 64], F32, tag="dT", bufs=2,
                                  name=f"dT{b}_{g}", uniquify=True)
                    nc.vector.tensor_tensor(out=dTg, in0=a3v[:, :, 0],
                                            in1=a3v[:, :, 1], op=ALU.subtract)
                    sTg = sm.tile([64, 128], F32, tag="sT", bufs=2,
                                  name=f"sT{b}_{g}", uniquify=True)
                    sTv = sTg.rearrange("p (a b) -> p a b", b=2)
                    nc.scalar.activation(out=sTv[:, :, 0], in_=dTg,
                                         func=AF.Sigmoid, scale=1.0)
                    nc.scalar.activation(out=sTv[:, :, 1], in_=dTg,
                                         func=AF.Sigmoid, scale=-1.0)
                    pst = psp.tile([128, 64], F32, tag="tp", bufs=1)
                    nc.tensor.transpose(pst, sTg, ident[0:64, 0:64])
                    sintg = sm.tile([128, 64], F32, tag="sint", bufs=4,
                                    name=f"sint{b}_{g}")
                    nc.scalar.copy(out=sintg, in_=pst)
                    # gate this group's y1 in place right away (pure multiply;
                    # the -m3 shift is const-folded into the final-evac bias).
                    # The gate row is fully materialized on gpsimd so the DVE
                    # multiply sees clean contiguous bf16 operands (2x mode).
                    y1 = S["y1"]
                    for hn in range(Hn):
                        gfull = sm.tile([128, WS, Wn, WS], BF16, tag="gfull",
                                        bufs=2, name=f"gf{b}_{g}_{hn}",
                                        uniquify=True)
                        gv = sintg[:, hn * Wn:(hn + 1) * Wn]
                        nc.gpsimd.tensor_copy(
                            out=gfull,
                            in_=gv.unsqueeze(1).unsqueeze(3).broadcast_to(
                                [128, WS, Wn, WS]))
                        nc.vector.tensor_tensor(
                            out=y1[g][:, bass.ts(hn, 512)],
                            in0=y1[g][:, bass.ts(hn, 512)],
                            in1=gfull.rearrange("p a b c -> p (a b c)"),
                            op=ALU.mult)

            def ph_final(b):
                """Final channel matmul; GN5 sums via scalar accumulation."""
                S = st[b]
                y1 = S["y1"]
                ot = [pf32.tile([128, NPIX], F32, tag="f32",
                                name=f"ot{b}_{i}") for i in range(2)]
                S["ot"] = ot
                s15 = [sm.tile([128, 4], F32, tag="s15", bufs=2,
                               name=f"s15_{b}_{i}") for i in range(2)]
                s25 = [sm.tile([128, 2], F32, tag="s25", bufs=2,
                               name=f"s25_{b}_{i}") for i in range(2)]
                for m in range(2):
                    for nq in range(4):
                        pacc = psp.tile([128, 1024], F32, tag="acc", bufs=3,
                                        name=f"pcf_{b}_{m}_{nq}",
                                        uniquify=True)
                        for ni in range(2):
                            n = nq * 2 + ni
                            for kc in range(4):
                                nc.tensor.matmul(
                                    pacc[:, ni * 512:(ni + 1) * 512],
                                    S["wds"][kc][:, m * 128:(m + 1) * 128],
                                    y1[kc][:, bass.ts(n, 512)],
                                    start=(kc == 0), stop=(kc == 3))
                        nsl = bass.ts(nq, 1024)
                        nc.scalar.activation(
                            out=ot[m][:, nsl], in_=pacc, func=AF.Identity,
                            bias=S["cmt"][m], scale=1.0,
                            accum_out=s15[m][:, nq:nq + 1])
                        if nq % 2 == 1:
                            sqd = sm.tile([128, 2048], BF16, tag="sqd",
                                          bufs=1, name=f"sqd5_{b}_{m}_{nq}",
                                          uniquify=True)
                            nc.scalar.activation(
                                out=sqd, in_=ot[m][:, bass.ts(nq // 2, 2048)],
                                func=AF.Square, scale=1.0,
                                accum_out=s25[m][:, nq // 2:nq // 2 + 1])
                S["s15"], S["s25"] = s15, s25

            def ph_out(b):
                """GN5 + residual + store."""
                S = st[b]
                mv5 = []
                for c in range(2):
                    mv = sm.tile([128, 2], F32, tag="mv5", bufs=2,
                                 name=f"mv5_{b}_{c}")
                    nc.vector.tensor_reduce(out=mv[:, 0:1], in_=S["s15"][c],
                                            axis=AX.X, op=ALU.add)
                    nc.vector.tensor_reduce(out=mv[:, 1:2], in_=S["s25"][c],
                                            axis=AX.X, op=ALU.add)
                    nc.vector.tensor_scalar(
                        out=mv, in0=mv, scalar1=1.0 / NPIX, scalar2=None,
                        op0=ALU.mult)
                    mv5.append(mv)
                sc5 = gn_scale_bias(mv5, gm1_t, rep1_t, 32, "gn5",
                                    raw_ex2=True)
                ov = out_d[b].rearrange("c h w -> c (h w)")
                hsv = hs[b].rearrange("c h w -> c (h w)")
                ot = S["ot"]
                for c in range(2):
                    for q in range(4):
                        qsl = bass.ts(q, NPIX // 4)
                        xrq = pxr.tile([128, NPIX // 4], F32, tag="xr",
                                       name=f"xr{b}_{c}_{q}", uniquify=True)
                        nc.sync.dma_start(
                            out=xrq, in_=hsv[c * 128:(c + 1) * 128, qsl])
                        if q % 2 == 0:
                            nc.gpsimd.tensor_scalar(
                                out=ot[c][:, qsl], in0=ot[c][:, qsl],
                                scalar1=sc5[c][:, 0:1], scalar2=sc5[c][:, 1:2],
                                op0=ALU.mult, op1=ALU.add)
                            nc.vector.tensor_tensor(out=xrq,
                                                    in0=ot[c][:, qsl],
                                                    in1=xrq, op=ALU.add)
                        else:
                            nc.vector.tensor_scalar(
                                out=ot[c][:, qsl], in0=ot[c][:, qsl],
                                scalar1=sc5[c][:, 0:1], scalar2=sc5[c][:, 1:2],
                                op0=ALU.mult, op1=ALU.add)
                            nc.gpsimd.tensor_tensor(out=xrq,
                                                    in0=ot[c][:, qsl],
                                                    in1=xrq, op=ALU.add)
                        nc.sync.dma_start(
                            out=ov[c * 128:(c + 1) * 128, qsl],
                            in_=xrq)

            # ------------------------------------------------ emission
            def scoped(name, fn, *a):
                s, _ = nc.enter_named_scope(name, False)
                fn(*a)
                nc.leave_named_scope(name, s, False)

            scoped("ld_0", ph_load, 0)
            scoped("conv0_0", ph_conv0, 0)
            scoped("ld_1", ph_load, 1)
            scoped("conv1_0", ph_conv1, 0, (0, 1, 2, 3))
            scoped("conv0_1", ph_conv0, 1)
            scoped("attn_0", ph_attn, 0)
            scoped("conv1_1a", ph_conv1, 1, (0,))
            scoped("final_0", ph_final, 0)
            scoped("conv1_1b", ph_conv1, 1, (1, 2, 3))
            scoped("out_0", ph_out, 0)
            scoped("attn_1", ph_attn, 1)
            scoped("final_1", ph_final, 1)
            scoped("out_1", ph_out, 1)

    nc.compile()
    return nc


# ---------------------------------------------------------------- entry

_CACHE = {}


def _get_nc(sim_safe=False):
    key = bool(sim_safe)
    if key not in _CACHE:
        _CACHE[key] = build_nc(sim_safe=key)
    return _CACHE[key]


def make_in_maps(inputs):
    hs_full = np.ascontiguousarray(inputs["hidden_state"], dtype=np.float32)
    wd = _host_weights(
        np.asarray(inputs["w0"], np.float32), np.asarray(inputs["b0"], np.float32),
        np.asarray(inputs["w1"], np.float32), np.asarray(inputs["b1"], np.float32),
        np.asarray(inputs["w2"], np.float32), np.asarray(inputs["b2"], np.float32),
        np.asarray(inputs["w3"], np.float32), np.asarray(inputs["b3"], np.float32),
        np.asarray(inputs["weight"], np.float32))
    cm = _host_consts()
    cpack, bpack = _pack_consts(wd, cm)
    assert cpack.shape[1] == NCF, cpack.shape
    assert bpack.shape[1] == NBF, bpack.shape
    shared = {"cpack": cpack, "bpack": bpack}
    in_maps = []
    for i in range(NCORES):
        m = dict(shared)
        m["hs"] = np.ascontiguousarray(hs_full[i * BPC:(i + 1) * BPC])
        m["hsb"] = m["hs"].astype(ml_dtypes.bfloat16)
        in_maps.append(m)
    return in_maps


def kernel(**inputs):
    from concourse import bass_utils
    nc = _get_nc(sim_safe=False)
    in_maps = make_in_maps(inputs)
    res = bass_utils.run_bass_kernel_spmd(nc, in_maps,
                                          core_ids=list(range(NCORES)))
    out = np.concatenate([res.results[i]["out"] for i in range(NCORES)], axis=0)
    return out.astype(np.float32)


# revision 47
# speedup vs baseline: 1.4008x; 1.4008x over previous
"""Trainium2 Bass kernel for nn_Block_16544214024520 (dense_cnn).

Data-parallel over batch: 16 samples -> 2 per NeuronCore x 8 cores.
All parameters replicated. Per-sample layout: channels on partitions
(256 = 2 chunks of 128), pixels (64x64 = 4096) on the free dim.

Reference pipeline (per sample):
  gn(32) -> 1x1 conv(256->256)+silu -> gn(16) -> 3x3 grouped conv
  (g=4, 256->512)+silu -> gn(2) -> window-mean(8x8) -> radix amax ->
  1x1 g-conv(256->64)+silu -> gn(8) -> 1x1 g-conv(64->512) ->
  softmax over radix(2) -> gated combine -> channel matmul(256->512?no 256)
  -> gn(32) -> +residual

Optimizations over the straightforward version:
  * conv1 (3x3 grouped) runs 5 matmuls per row-tile instead of 18
    half-width ones: (ky=0,ky=1) taps pair into K=128 matmuls via a
    row-shifted duplicate of the padded input (SBUF->SBUF DMA), and
    (ky=2,kx=0,1) pair via a col-shifted duplicate; one K=64 single
    remains. Each weight block is reused across 4 row-tiles.
  * conv0/conv1/final accumulate into [128,1024] PSUM tiles (2 banks)
    -> half the scalar-engine evacuations.
  * GN3/GN5 statistics come from the scalar engine: means accumulate
    for free on the evacuation (accum_out), E[x^2] via a Square pass,
    replacing ~70us of DVE bn_stats.
  * radix softmax gating is a pure bf16 multiply: since a0+a1==1, the
    -mean3 shift const-folds into the final-evacuation bias.
  * window-pool partials in one tensor_reduce per group; per-group
    conv3->sigmoid->gate pipeline so gating starts early.
  * residual is loaded just-in-time in [128,1024] quarters; GN5 apply
    and the residual add alternate between gpsimd and DVE.
  * two samples emitted with a skewed, split-conv1 schedule so the
    second sample's conv1 fills the first sample's attention phase.
"""

import os
import sys

for _p in ("/opt/trn_rl_repo", "/opt/pypackages"):
    if _p not in sys.path:
        sys.path.append(_p)

import ml_dtypes
import numpy as np

import concourse.bass as bass  # noqa: F401
import concourse.mybir as mybir
import concourse.tile as tile
from concourse import bacc
from concourse.masks import make_identity

F32 = mybir.dt.float32
BF16 = mybir.dt.bfloat16
AF = mybir.ActivationFunctionType
ALU = mybir.AluOpType
AX = mybir.AxisListType

NCORES = 8
BPC = 2          # samples per core
C = 256          # channels
H = W = 64
NPIX = H * W     # 4096
PADW = W + 2     # 66
Hn = Wn = 8      # window grid
WS = 8           # window size
EPS = 1e-5
NT = 8           # n-tiles of 512 pixels (8 rows of 64)
XGROWS = 65      # rows in the shifted-dup conv1 input buffer


# ---------------------------------------------------------------- host prep

def _host_consts():
    """Constant matrices shared by all cores (built once)."""
    c = {}
    # GN over 256 channels, 32 groups of 8 (GN1/GN5)
    gm1 = np.zeros((2, 128, 32), np.float32)
    rep1 = np.zeros((2, 128, 128), np.float32)
    for ch in range(2):
        for k in range(128):
            g = (128 * ch + k) // 8
            gm1[ch, k, g] = 1.0 / 8.0
        for m in range(128):
            rep1[ch, (128 * ch + m) // 8 % 128, m] = 1.0
    c["gm1"] = gm1
    c["rep1"] = rep1
    # GN2: 16 groups of 16 over 256 channels
    gm2 = np.zeros((2, 128, 16), np.float32)
    rep2 = np.zeros((2, 128, 128), np.float32)
    for ch in range(2):
        for k in range(128):
            gm2[ch, k, (128 * ch + k) // 16] = 1.0 / 16.0
        for m in range(128):
            rep2[ch, (128 * ch + m) // 16, m] = 1.0
    c["gm2"] = gm2
    c["rep2"] = rep2
    # GN3 over 512 channels, 2 groups of 256 (chunks 0,1 -> g0; 2,3 -> g1)
    g3 = np.zeros((4, 128, 2), np.float32)
    r3 = np.zeros((4, 128, 128), np.float32)
    for mc in range(4):
        g3[mc, :, mc // 2] = 1.0 / 256.0
        r3[mc, mc // 2, :] = 1.0
    c["g3"] = g3
    c["r3"] = r3
    # GN4 over 64 channels, 8 groups of 8
    g4 = np.zeros((128, 8), np.float32)
    for k in range(64):
        g4[k, k // 8] = 1.0 / 8.0
    r4 = np.zeros((128, 64), np.float32)
    for m in range(64):
        r4[m // 8, m] = 1.0
    c["g4"] = g4
    c["r4"] = r4
    return c


def _host_weights(w0, b0, w1, b1, w2, b2, w3, b3, weight):
    """Rearrange torch-layout conv weights into matmul lhsT tensors."""
    d = {}
    # conv0: out[o,p] = sum_i w0[o,i] x[i,p]  -> lhsT[i,o]
    d["w0T"] = np.ascontiguousarray(w0[:, :, 0, 0].T).astype(
        ml_dtypes.bfloat16)  # [256,256]
    d["b0c"] = np.ascontiguousarray(b0.reshape(C, 1)).astype(np.float32)
    # conv1: grouped 3x3, groups=4 (in 64 -> out 128 each).
    # Row-pair lhsT per (g, dx): [128,128] rows 0:64 = ky=0, rows 64:128 =
    # ky=1 (paired via the row-shifted dup buffer xg).
    # Col-pair lhsT per g: rows 0:64 = (ky=2, kx=0), rows 64:128 =
    # (ky=2, kx=1) (paired via the col-shifted dup buffer xh).
    # Last single per g: rows 0:64 = (ky=2, kx=2).
    w1p = np.zeros((4, 3, 128, 128), np.float32)
    w1c = np.zeros((4, 128, 128), np.float32)
    w1e = np.zeros((4, 128, 128), np.float32)
    for g in range(4):
        for dx in range(3):
            w1p[g, dx, 0:64, :] = w1[g * 128:(g + 1) * 128, :, 0, dx].T
            w1p[g, dx, 64:128, :] = w1[g * 128:(g + 1) * 128, :, 1, dx].T
        w1c[g, 0:64, :] = w1[g * 128:(g + 1) * 128, :, 2, 0].T
        w1c[g, 64:128, :] = w1[g * 128:(g + 1) * 128, :, 2, 1].T
        w1e[g, 0:64, :] = w1[g * 128:(g + 1) * 128, :, 2, 2].T
    d["w1p"] = w1p.astype(ml_dtypes.bfloat16)
    d["w1c"] = w1c.astype(ml_dtypes.bfloat16)
    d["w1e"] = w1e.astype(ml_dtypes.bfloat16)
    d["b1c"] = np.ascontiguousarray(b1.reshape(2 * C, 1)).astype(np.float32)
    # conv2: groups=2 (in 128 -> out 32)
    w2t = np.zeros((2, 128, 32), np.float32)
    for g in range(2):
        w2t[g] = w2[g * 32:(g + 1) * 32, :, 0, 0].T
    d["w2t"] = w2t
    d["b2c"] = np.ascontiguousarray(b2.reshape(64, 1)).astype(np.float32)
    # conv3: groups=2 (in 32 -> out 256); K padded to 128 with zero rows.
    w3t = np.zeros((4, 128, 128), np.float32)
    for g in range(4):
        src = w3[g * 128:(g + 1) * 128, :, 0, 0]      # [128, 32]
        r0 = 0 if g < 2 else 32
        w3t[g, r0:r0 + 32, :] = src.T
    d["w3t"] = w3t
    # final einsum: out[c,p] = sum_C weight[C,c] z[C,p], z[C] = zint[2C]+zint[2C+1]
    # fold the radix pair-sum by duplicating rows: wdup[c512, c] = weight[c512//2, c]
    wdup = np.repeat(weight.astype(np.float32), 2, axis=0)   # [512, 256]
    d["wdupT"] = np.ascontiguousarray(wdup).astype(ml_dtypes.bfloat16)
    return d


def _pack_consts(wd, cm):
    """Pack all fp32 constants into one [128, F] tensor and all bf16
    weights into another, so startup needs only two DMAs."""
    fcols = []   # list of [128, n] fp32 blocks
    def addf(x):
        x = np.asarray(x, np.float32)
        assert x.shape[0] == 128
        fcols.append(x.reshape(128, -1))
    for c in range(2):
        addf(cm["gm1"][c]); addf(cm["rep1"][c])
        addf(cm["gm2"][c]); addf(cm["rep2"][c])
    for g in range(4):
        addf(cm["g3"][g]); addf(cm["r3"][g])
    addf(cm["g4"]); addf(cm["r4"])
    b0 = wd["b0c"].reshape(2, 128, 1)
    addf(b0[0]); addf(b0[1])
    b1 = wd["b1c"].reshape(4, 128, 1)
    for g in range(4):
        addf(b1[g])
    b2p = np.zeros((128, 1), np.float32)
    b2p[0:64] = wd["b2c"]
    addf(b2p)
    addf(np.full((128, 1), EPS, np.float32))
    for g in range(2):
        addf(wd["w2t"][g])
    for g in range(4):
        addf(wd["w3t"][g])
    cpack = np.concatenate(fcols, axis=1)

    w0 = np.asarray(wd["w0T"])
    bcols = [w0[0:128], w0[128:256]]
    for g in range(4):
        for dx in range(3):
            bcols.append(np.asarray(wd["w1p"])[g, dx])
        bcols.append(np.asarray(wd["w1c"])[g])
        bcols.append(np.asarray(wd["w1e"])[g])
    wdp = np.asarray(wd["wdupT"])
    for k in range(4):
        bcols.append(wdp[k * 128:(k + 1) * 128])
    bpack = np.concatenate(bcols, axis=1).astype(ml_dtypes.bfloat16)
    return cpack, bpack


NCF = 32 + 128 + 16 + 128 + 32 + 128 + 16 + 128 + 4 * (2 + 128) \
    + 8 + 64 + 2 + 4 + 1 + 1 + 2 * 32 + 4 * 128
NBF = 256 * 2 + 4 * 5 * 128 + 4 * 256


# ---------------------------------------------------------------- builder

def build_nc(sim_safe: bool = False):
    nc = bacc.Bacc("TRN2", target_bir_lowering=False, debug=False,
                   num_devices=NCORES)

    def din(name, shape, dt=F32):
        return nc.dram_tensor(name, list(shape), dt, kind="ExternalInput").ap()

    hs = din("hs", (BPC, C, H, W))
    hsb = din("hsb", (BPC, C, H, W), BF16)
    cpack_d = din("cpack", (128, NCF))
    bpack_d = din("bpack", (128, NBF), BF16)

    out_d = nc.dram_tensor("out", [BPC, C, H, W], F32, kind="ExternalOutput").ap()

    with tile.TileContext(nc) as tc:
        with tc.tile_pool(name="consts", bufs=1) as cst, \
             tc.tile_pool(name="b16", bufs=8) as pb16, \
             tc.tile_pool(name="xq", bufs=2) as pxq, \
             tc.tile_pool(name="xg", bufs=2) as pxg, \
             tc.tile_pool(name="xh", bufs=2) as pxh, \
             tc.tile_pool(name="f32", bufs=2) as pf32, \
             tc.tile_pool(name="xr", bufs=3) as pxr, \
             tc.tile_pool(name="small", bufs=2) as sm, \
             tc.tile_pool(name="psum", bufs=2, space="PSUM") as psp:

            # ---- load constants / weights (two packed DMAs) ----
            cpk = cst.tile([128, NCF], F32, name="cpk")
            nc.sync.dma_start(out=cpk, in_=cpack_d)
            bpk = cst.tile([128, NBF], BF16, name="bpk")
            nc.sync.dma_start(out=bpk, in_=bpack_d)

            class _Cur:
                def __init__(self):
                    self.o = 0
            _cf, _cb = _Cur(), _Cur()

            def fsl(n):
                s = cpk[:, _cf.o:_cf.o + n]
                _cf.o += n
                return s

            def bsl(n):
                s = bpk[:, _cb.o:_cb.o + n]
                _cb.o += n
                return s

            gm1_t, rep1_t, gm2_t, rep2_t = [], [], [], []
            for c in range(2):
                gm1_t.append(fsl(32)); rep1_t.append(fsl(128))
                gm2_t.append(fsl(16)); rep2_t.append(fsl(128))
            g3_t, r3_t = [], []
            for g in range(4):
                g3_t.append(fsl(2)); r3_t.append(fsl(128))
            g4_t = fsl(8); r4_t = fsl(64)
            b0_t = [fsl(1) for _ in range(2)]
            b1_t = [fsl(1) for _ in range(4)]
            b2_t = fsl(1)
            eps_t = fsl(1)
            w2_t = [fsl(32) for _ in range(2)]
            w3_t = [fsl(128) for _ in range(4)]
            assert _cf.o == NCF
            w0_t = [bsl(256) for _ in range(2)]
            w1p_t = [[None] * 3 for _ in range(4)]
            w1c_t = [None] * 4
            w1e_t = [None] * 4
            for g in range(4):
                for dx in range(3):
                    w1p_t[g][dx] = bsl(128)
                w1c_t[g] = bsl(128)
                w1e_t[g] = bsl(128)
            wd_t = [bsl(256) for _ in range(4)]
            assert _cb.o == NBF
            ident = cst.tile([128, 128], F32, name="ident")
            make_identity(nc, ident)

            # ------------------------------------------------ helpers
            def silu_evac(out_ap, psum_ap, bias_ap, tag, accum_out=None):
                """out = silu(psum + bias); fused on HW, 2-op in CoreSim."""
                if not sim_safe:
                    nc.scalar.activation(out=out_ap, in_=psum_ap, func=AF.Silu,
                                         bias=bias_ap, scale=1.0,
                                         accum_out=accum_out)
                    return
                if True:
                    ff = psum_ap.free_size()
                    pp = psum_ap.partition_size()
                    sgf = sm.tile([128, 1024], F32, tag="sg", bufs=1,
                                  name=f"sg_{tag}", uniquify=True)
                    sgt = sgf[0:pp, 0:ff]
                    nc.scalar.activation(out=sgt, in_=psum_ap, func=AF.Sigmoid,
                                         bias=bias_ap, scale=1.0)
                    nc.vector.scalar_tensor_tensor(
                        out=out_ap, in0=psum_ap, scalar=bias_ap, in1=sgt,
                        op0=ALU.add, op1=ALU.mult)
                    if accum_out is not None:
                        nc.scalar.activation(out=sgt, in_=out_ap,
                                             func=AF.Identity, scale=1.0,
                                             accum_out=accum_out)

            def gn_scale_bias(mvs, gmat_list, rmat_list, ngroups, tag,
                              ncols=2, raw_ex2=False):
                """Per-channel (scale, bias) tiles for a group norm.

                mvs: list of [128, 2] SBUF tiles of per-channel (mean, var).
                Returns list of [128, ncols] tiles (col0 = rstd,
                col1 = -mean*rstd, col2 = -mean) replicated back to channels.
                """
                nchunk = len(mvs)
                if raw_ex2:
                    # mvs are already [128, 2] = (mean, E[x^2]) tiles
                    rstats = mvs
                else:
                    rstats = []
                    for ci, mv in enumerate(mvs):
                        r = sm.tile([128, 2], F32, tag=f"r_{tag}",
                                    bufs=2 * nchunk)
                        nc.vector.tensor_copy(out=r[:, 0:1], in_=mv[:, 0:1])
                        nc.vector.scalar_tensor_tensor(
                            out=r[:, 1:2], in0=mv[:, 0:1], scalar=mv[:, 0:1],
                            in1=mv[:, 1:2], op0=ALU.mult, op1=ALU.add)
                        rstats.append(r)
                pg = psp.tile([128, 2], F32, tag="gn_ps", bufs=1)
                for ci in range(nchunk):
                    nc.tensor.matmul(pg[0:ngroups, :], gmat_list[ci], rstats[ci],
                                     start=(ci == 0), stop=(ci == nchunk - 1))
                gt = sm.tile([128, 2], F32, tag=f"gt_{tag}", bufs=2)
                nc.vector.memset(gt, 0.0)
                nc.scalar.copy(out=gt[0:ngroups, :], in_=pg[0:ngroups, :])
                # -var = mean^2 - E[x^2]
                negv = sm.tile([128, 1], F32, tag=f"nv_{tag}", bufs=2)
                nc.vector.scalar_tensor_tensor(
                    out=negv[0:ngroups], in0=gt[0:ngroups, 0:1],
                    scalar=gt[0:ngroups, 0:1], in1=gt[0:ngroups, 1:2],
                    op0=ALU.mult, op1=ALU.subtract)
                sd = sm.tile([128, 1], F32, tag=f"sd_{tag}", bufs=2)
                nc.scalar.activation(out=sd[0:ngroups], in_=negv[0:ngroups],
                                     func=AF.Sqrt, bias=eps_t[0:ngroups],
                                     scale=-1.0)
                rstd = sm.tile([128, 1], F32, tag=f"rs_{tag}", bufs=2)
                nc.vector.reciprocal(out=rstd[0:ngroups], in_=sd[0:ngroups])
                stg = sm.tile([128, 3], F32, tag=f"st_{tag}", bufs=2)
                nc.vector.memset(stg, 0.0)
                nc.vector.tensor_copy(out=stg[0:ngroups, 0:1], in_=rstd[0:ngroups])
                nc.vector.tensor_scalar(
                    out=stg[0:ngroups, 1:2], in0=gt[0:ngroups, 0:1],
                    scalar1=rstd[0:ngroups], scalar2=-1.0,
                    op0=ALU.mult, op1=ALU.mult)
                if ncols == 3:
                    nc.vector.tensor_scalar(
                        out=stg[0:ngroups, 2:3], in0=gt[0:ngroups, 0:1],
                        scalar1=-1.0, scalar2=None, op0=ALU.mult)
                scs = []
                for ci, rmat in enumerate(rmat_list):
                    mm = rmat.shape[-1]
                    pr = psp.tile([128, 3], F32, tag="gn_ps", bufs=1)
                    nc.tensor.matmul(pr[0:mm, 0:ncols], rmat,
                                     stg[:, 0:ncols], start=True, stop=True)
                    sc = sm.tile([128, 3], F32, tag=f"sc_{tag}",
                                 bufs=2 * nchunk)
                    nc.scalar.copy(out=sc[0:mm, 0:ncols], in_=pr[0:mm, 0:ncols])
                    scs.append(sc)
                return scs

            st = [dict() for _ in range(BPC)]

            # ------------------------------------------------ phases
            def ph_load(b):
                """Load input, GN1 stats, fold GN1 into conv0 weights."""
                S = st[b]
                hsbv = hsb[b].rearrange("c h w -> c (h w)")
                S["xw"] = [pb16.tile([128, NPIX], BF16, tag="b16",
                                     name=f"xw{b}_{i}") for i in range(2)]
                bst1 = [sm.tile([128, NT, 6], F32, tag="bst1", bufs=2,
                                name=f"bst1_{b}_{i}") for i in range(2)]
                for c in range(2):
                    nc.sync.dma_start(out=S["xw"][c],
                                      in_=hsbv[c * 128:(c + 1) * 128, :])
                    for n in range(NT):
                        nc.vector.bn_stats(out=bst1[c][:, n, :],
                                           in_=S["xw"][c][:, bass.ts(n, 512)])
                mv1 = []
                for c in range(2):
                    mv = sm.tile([128, 2], F32, tag="mv1", bufs=2,
                                 name=f"mv1_{b}_{c}")
                    nc.vector.bn_aggr(out=mv, in_=bst1[c])
                    mv1.append(mv)
                sc1 = gn_scale_bias(mv1, gm1_t, rep1_t, 32, "gn1")
                # fold GN1 into conv0 weights
                w0s = [sm.tile([128, 256], BF16, tag="w0s", bufs=2,
                               name=f"w0s{b}_{i}") for i in range(2)]
                t1b = [sm.tile([128, 1], BF16, tag="t1b", bufs=2,
                               name=f"t1b{b}_{i}") for i in range(2)]
                for c in range(2):
                    nc.vector.tensor_scalar_mul(out=w0s[c], in0=w0_t[c],
                                                scalar1=sc1[c][:, 0:1])
                    nc.vector.tensor_copy(out=t1b[c], in_=sc1[c][:, 1:2])
                b0p = [sm.tile([128, 1], F32, tag="b0p", bufs=2,
                               name=f"b0p{b}_{i}") for i in range(2)]
                for m in range(2):
                    pb = psp.tile([128, 1], F32, tag="gn_ps", bufs=1)
                    for kc in range(2):
                        nc.tensor.matmul(
                            pb,
                            w0s[kc][:, m * 128:(m + 1) * 128],
                            t1b[kc],
                            start=(kc == 0), stop=(kc == 1))
                    nc.scalar.activation(out=b0p[m], in_=pb,
                                         func=AF.Identity, bias=b0_t[m],
                                         scale=1.0)
                S["w0s"] = w0s
                S["b0p"] = b0p

            def ph_conv0(b):
                """conv0 (1x1)+silu straight into padded conv1 input; GN2
                stats + in-place apply; build shifted-dup buffers by DMA."""
                S = st[b]
                xq = [pxq.tile([128, PADW, PADW], BF16, tag="xq",
                               name=f"xq{b}_{i}") for i in range(2)]
                S["xq"] = xq
                for c in range(2):
                    xpf = xq[c]
                    nc.gpsimd.memset(xpf[:, 0:1, :], 0.0)
                    nc.gpsimd.memset(xpf[:, PADW - 1:PADW, :], 0.0)
                    nc.gpsimd.memset(xpf[:, 1:PADW - 1, 0:1], 0.0)
                    nc.gpsimd.memset(xpf[:, 1:PADW - 1, PADW - 1:PADW], 0.0)
                y0 = [pb16.tile([128, NPIX], BF16, tag="b16",
                                name=f"y0{b}_{i}") for i in range(2)]
                bst2 = [sm.tile([128, NT, 6], F32, tag="bst2", bufs=2,
                                name=f"bst2_{b}_{i}") for i in range(2)]
                for m in range(2):
                    for nq in range(4):
                        pacc = psp.tile([128, 1024], F32, tag="acc", bufs=3,
                                        name=f"pc0_{b}_{m}_{nq}",
                                        uniquify=True)
                        for ni in range(2):
                            n = nq * 2 + ni
                            for kc in range(2):
                                nc.tensor.matmul(
                                    pacc[:, ni * 512:(ni + 1) * 512],
                                    S["w0s"][kc][:, m * 128:(m + 1) * 128],
                                    S["xw"][kc][:, bass.ts(n, 512)],
                                    start=(kc == 0), stop=(kc == 1))
                        nsl = bass.ts(nq, 1024)
                        silu_evac(y0[m][:, nsl], pacc, S["b0p"][m],
                                  f"c0_{b}")
                        for ni in range(2):
                            n = nq * 2 + ni
                            nc.vector.bn_stats(out=bst2[m][:, n, :],
                                               in_=y0[m][:, bass.ts(n, 512)])
                mv2 = []
                for c in range(2):
                    mv = sm.tile([128, 2], F32, tag="mv2", bufs=2,
                                 name=f"mv2_{b}_{c}")
                    nc.vector.bn_aggr(out=mv, in_=bst2[c])
                    mv2.append(mv)
                sc2 = gn_scale_bias(mv2, gm2_t, rep2_t, 16, "gn2")
                for c in range(2):
                    nc.gpsimd.tensor_scalar(
                        out=xq[c][:, 1:65, 1:65],
                        in0=y0[c].rearrange("p (h w) -> p h w", h=H),
                        scalar1=sc2[c][:, 0:1], scalar2=sc2[c][:, 1:2],
                        op0=ALU.mult, op1=ALU.add)
                # shifted-dup buffers for conv1 tap pairing:
                # parts 0:64 <- xq rows 0..64 (offset r*66 holds image row r-1)
                # parts 64:128 <- xq rows 1..65 (offset r*66 holds image row r)
                xg = [pxg.tile([128, XGROWS, PADW], BF16, tag="xg",
                               name=f"xg{b}_{g}") for g in range(4)]
                xh = [pxh.tile([128, XGROWS, PADW], BF16, tag="xh",
                               name=f"xh{b}_{g}") for g in range(4)]
                S["xg"] = xg
                S["xh"] = xh
                for g in range(4):
                    kc, blk = g // 2, g % 2
                    src = xq[kc]
                    p0 = blk * 64
                    nc.sync.dma_start(
                        out=xg[g][0:64, :, :],
                        in_=src[p0:p0 + 64, 0:XGROWS, :])
                    nc.sync.dma_start(
                        out=xg[g][64:128, :, :],
                        in_=src[p0:p0 + 64, 1:1 + XGROWS, :])
                    nc.sync.dma_start(
                        out=xh[g][0:64, :, :],
                        in_=src[p0:p0 + 64, 1:1 + XGROWS, :])
                    nc.sync.dma_start(
                        out=xh[g][64:128, :, 0:PADW - 1],
                        in_=src[p0:p0 + 64, 1:1 + XGROWS, 1:PADW])

            def ph_conv1(b, gs):
                """conv1 (3x3 grouped, tap-paired) + silu -> y1 for groups
                in gs; GN3 stats and window-pool partials in-loop."""
                S = st[b]
                if 0 in gs:
                    S["y1"] = [pb16.tile([128, NPIX], BF16, tag="b16",
                                         name=f"y1{b}_{g}") for g in range(4)]
                    S["s13"] = [sm.tile([128, 4], F32, tag="s13", bufs=4,
                                        name=f"s13_{b}_{g}") for g in range(4)]
                    S["s23"] = [sm.tile([128, 2], F32, tag="s23", bufs=4,
                                        name=f"s23_{b}_{g}") for g in range(4)]
                    S["amT"] = sm.tile([64, 256], F32, tag="amT", bufs=1,
                                       name=f"amT{b}")
                y1, s13, s23, amT = S["y1"], S["s13"], S["s23"], S["amT"]
                for g in gs:
                    pa_g = sm.tile([128, NT * 64], BF16, tag="pa", bufs=2,
                                   name=f"pa{b}_{g}")
                    xgv = S["xg"][g]
                    xhv = S["xh"][g]
                    for np2 in range(2):
                        paccs = [psp.tile([128, 1024], F32, tag="acc", bufs=3,
                                          name=f"pc1_{b}_{g}_{np2}_{t}",
                                          uniquify=True) for t in range(2)]
                        # each weight block is loaded once and streamed over
                        # 4 row-tiles (2 psum tiles x 2 halves)
                        for dx in range(3):
                            for t in range(2):
                                for ni in range(2):
                                    n = (np2 * 2 + t) * 2 + ni
                                    r0 = n * WS
                                    nc.tensor.matmul(
                                        paccs[t][:, ni * 512:(ni + 1) * 512],
                                        w1p_t[g][dx],
                                        xgv[:, r0:r0 + 8, dx:dx + 64],
                                        start=(dx == 0), stop=False)
                        for t in range(2):
                            for ni in range(2):
                                n = (np2 * 2 + t) * 2 + ni
                                r0 = n * WS
                                nc.tensor.matmul(
                                    paccs[t][:, ni * 512:(ni + 1) * 512],
                                    w1c_t[g],
                                    xhv[:, r0 + 1:r0 + 9, 0:64],
                                    start=False, stop=False)
                        for t in range(2):
                            for ni in range(2):
                                n = (np2 * 2 + t) * 2 + ni
                                r0 = n * WS
                                nc.tensor.matmul(
                                    paccs[t][:, ni * 512:(ni + 1) * 512],
                                    w1e_t[g][0:64, :],
                                    xhv[0:64, r0 + 1:r0 + 9, 2:66],
                                    start=False, stop=True)
                        for t in range(2):
                            npair = np2 * 2 + t
                            nsl = bass.ts(npair, 1024)
                            silu_evac(y1[g][:, nsl], paccs[t], b1_t[g],
                                      f"c1_{b}",
                                      accum_out=s13[g][:, npair:npair + 1])
                        sqd = sm.tile([128, 2048], BF16, tag="sqd",
                                      bufs=1, name=f"sqd3_{b}_{g}_{np2}",
                                      uniquify=True)
                        nc.scalar.activation(
                            out=sqd, in_=y1[g][:, bass.ts(np2, 2048)],
                            func=AF.Square, scale=1.0,
                            accum_out=s23[g][:, np2:np2 + 1])
                    # window-pool partials in one reduce per group
                    with nc.allow_low_precision(reason="bf16 pool partials"):
                        nc.vector.tensor_reduce(
                            out=pa_g,
                            in_=y1[g].rearrange("p (a w2) -> p a w2", w2=WS),
                            axis=AX.X, op=ALU.add)
                    # finish this group's window means + transpose + radix max
                    pooled = sm.tile([128, Hn, Wn], F32, tag="pooled", bufs=2,
                                     name=f"pooled{b}_{g}", uniquify=True)
                    pav = pa_g.rearrange("p (hn h2 wn) -> p hn wn h2",
                                         hn=Hn, h2=WS)
                    nc.vector.tensor_reduce(out=pooled, in_=pav,
                                            axis=AX.X, op=ALU.add)
                    ptp = psp.tile([64, 128], F32, tag="tp", bufs=1)
                    nc.tensor.transpose(
                        ptp, pooled.rearrange("p a b -> p (a b)"), ident)
                    pooledT = sm.tile([64, 128], F32, tag="pooledT", bufs=2,
                                      name=f"pooledT{b}_{g}", uniquify=True)
                    nc.scalar.copy(out=pooledT, in_=ptp)
                    pv = pooledT.rearrange("p (a b) -> p a b", b=2)
                    nc.vector.tensor_tensor(
                        out=amT[:, g * 64:(g + 1) * 64],
                        in0=pv[:, :, 0], in1=pv[:, :, 1], op=ALU.max)
                if 3 not in gs:
                    return
                mv3 = []
                for g in range(4):
                    mv = sm.tile([128, 2], F32, tag="mv3", bufs=4,
                                 name=f"mv3_{b}_{g}")
                    nc.vector.tensor_reduce(out=mv[:, 0:1], in_=s13[g],
                                            axis=AX.X, op=ALU.add)
                    nc.vector.tensor_reduce(out=mv[:, 1:2], in_=s23[g],
                                            axis=AX.X, op=ALU.add)
                    nc.vector.tensor_scalar(
                        out=mv, in0=mv, scalar1=1.0 / NPIX, scalar2=None,
                        op0=ALU.mult)
                    mv3.append(mv)
                sc3 = gn_scale_bias(mv3, g3_t, r3_t, 2, "gn3", ncols=3,
                                    raw_ex2=True)
                S["sc3"] = sc3
                # fold GN3 scale into the final matmul weights
                wds = [sm.tile([128, 256], BF16, tag="wds", bufs=4,
                               name=f"wds{b}_{kc}") for kc in range(4)]
                t3b = [sm.tile([128, 1], BF16, tag="t3b", bufs=4,
                               name=f"t3b{b}_{kc}") for kc in range(4)]
                for kc in range(4):
                    nc.vector.tensor_scalar_mul(
                        out=wds[kc], in0=wd_t[kc],
                        scalar1=sc3[kc][:, 0:1])
                    # 0.5: the wdup row-duplication would count m3 twice
                    nc.vector.tensor_scalar(
                        out=t3b[kc], in0=sc3[kc][:, 2:3], scalar1=0.5,
                        scalar2=None, op0=ALU.mult)
                # const-fold: cm[co] = sum_C wds[C,co] * (-m3[C]); becomes
                # the final-evac bias (valid because a0 + a1 == 1).
                cmt = [sm.tile([128, 1], F32, tag="cmt", bufs=2,
                               name=f"cmt{b}_{m}") for m in range(2)]
                for m in range(2):
                    pcm = psp.tile([128, 1], F32, tag="gn_ps", bufs=1)
                    for kc in range(4):
                        nc.tensor.matmul(
                            pcm, wds[kc][:, m * 128:(m + 1) * 128], t3b[kc],
                            start=(kc == 0), stop=(kc == 3))
                    nc.scalar.copy(out=cmt[m], in_=pcm)
                S["cmt"] = cmt
                S["wds"] = wds

            def ph_attn(b):
                """Window mean finish, radix amax, conv2+GN4+conv3,
                softmax -> per-group gate tiles; also load the residual."""
                S = st[b]
                sc3 = S["sc3"]
                amT = S["amT"]
                am = [sm.tile([128, 64], F32, tag="am", bufs=2,
                              name=f"am{b}_{i}") for i in range(2)]
                s64 = [sm.tile([128, 1], F32, tag="s64", bufs=2,
                               name=f"s64_{b}_{i}") for i in range(2)]
                for c in range(2):
                    pta = psp.tile([128, 64], F32, tag="tp", bufs=1)
                    nc.tensor.transpose(pta, amT[:, c * 128:(c + 1) * 128],
                                        ident[0:64, 0:64])
                    nc.scalar.copy(out=am[c], in_=pta)
                    # normalize the pooled maxima: am = am*(s3/64) + t3
                    nc.vector.tensor_scalar(
                        out=s64[c], in0=sc3[2 * c][:, 0:1],
                        scalar1=1.0 / (WS * WS), scalar2=None, op0=ALU.mult)
                    nc.vector.tensor_scalar(
                        out=am[c], in0=am[c], scalar1=s64[c],
                        scalar2=sc3[2 * c][:, 1:2], op0=ALU.mult, op1=ALU.add)

                # ---- conv2 (1x1 g=2, 256->64) + silu ----
                p2 = psp.tile([128, 64], F32, tag="tp", bufs=1)
                for g in range(2):
                    nc.tensor.matmul(p2[g * 32:(g + 1) * 32, :], w2_t[g], am[g],
                                     start=True, stop=True)
                a2 = sm.tile([128, 64], F32, tag="a2", bufs=2)
                nc.vector.memset(a2, 0.0)
                silu_evac(a2[0:64, :], p2[0:64, :], b2_t[0:64], f"c2_{b}")

                # ---- GN4 -> a2n ----
                mv4pad = sm.tile([128, 2], F32, tag="mv4", bufs=2)
                nc.vector.memset(mv4pad, 0.0)
                bst4 = sm.tile([128, 1, 6], F32, tag="bst4", bufs=2)
                nc.vector.bn_stats(out=bst4[0:64], in_=a2[0:64].unsqueeze(1))
                nc.vector.bn_aggr(out=mv4pad[0:64], in_=bst4[0:64])
                sc4 = gn_scale_bias([mv4pad], [g4_t], [r4_t], 8, "gn4")[0]
                a2n = sm.tile([128, 64], F32, tag="a2n", bufs=2)
                nc.vector.memset(a2n, 0.0)
                nc.vector.tensor_scalar(
                    out=a2n[0:64], in0=a2[0:64],
                    scalar1=sc4[0:64, 0:1], scalar2=sc4[0:64, 1:2],
                    op0=ALU.mult, op1=ALU.add)

                # ---- conv3 (1x1 g=2, 64->512), b3 = 0; then softmax over
                # radix == sigmoid of pair difference; fully per-group so the
                # first gate tile is ready early ----
                grow = [sm.tile([128, Hn, Wn, WS], BF16, tag="grow", bufs=4,
                                name=f"grow{b}_{g}") for g in range(4)]
                for g in range(4):
                    p3 = psp.tile([128, 64], F32, tag="tp", bufs=1)
                    nc.tensor.matmul(p3, w3_t[g], a2n, start=True, stop=True)
                    a3 = sm.tile([128, 64], F32, tag="a3", bufs=2)
                    nc.scalar.copy(out=a3, in_=p3)
                    p3t = psp.tile([64, 128], F32, tag="tp", bufs=1)
                    nc.tensor.transpose(p3t, a3, ident)
                    a3Tg = sm.tile([64, 128], F32, tag="a3T", bufs=2,
                                   name=f"a3T{b}_{g}", uniquify=True)
                    nc.scalar.copy(out=a3Tg, in_=p3t)
                    a3v = a3Tg.rearrange("p (a b) -> p a b", b=2)
                    dTg = sm.tile([64, 64], F32, tag="dT", bufs=2,
                                  name=f"dT{b}_{g}", uniquify=True)
                    nc.vector.tensor_tensor(out=dTg, in0=a3v[:, :, 0],
                                            in1=a3v[:, :, 1], op=ALU.subtract)
                    sTg = sm.tile([64, 128], F32, tag="sT", bufs=2,
                                  name=f"sT{b}_{g}", uniquify=True)
                    sTv = sTg.rearrange("p (a b) -> p a b", b=2)
                    nc.scalar.activation(out=sTv[:, :, 0], in_=dTg,
                                         func=AF.Sigmoid, scale=1.0)
                    nc.scalar.activation(out=sTv[:, :, 1], in_=dTg,
                                         func=AF.Sigmoid, scale=-1.0)
                    pst = psp.tile([128, 64], F32, tag="tp", bufs=1)
                    nc.tensor.transpose(pst, sTg, ident[0:64, 0:64])
                    sintg = sm.tile([128, 64], F32, tag="sint", bufs=4,
                                    name=f"sint{b}_{g}")
                    nc.scalar.copy(out=sintg, in_=pst)
                    gv = sintg.rearrange("p (hn wn) -> p hn wn", hn=Hn)
                    nc.gpsimd.tensor_copy(
                        out=grow[g],
                        in_=gv.unsqueeze(3).broadcast_to([128, Hn, Wn, WS]))
                    # gate this group's y1 in place right away (pure multiply;
                    # the -m3 shift is const-folded into the final-evac bias)
                    y1 = S["y1"]
                    for hn in range(Hn):
                        gsl = grow[g][:, hn, :, :].rearrange("p a b -> p (a b)")
                        yv = y1[g][:, bass.ts(hn, 512)].rearrange(
                            "p (h2 x) -> p h2 x", h2=WS)
                        eng = nc.gpsimd if hn % 4 == 3 else nc.vector
                        eng.tensor_tensor(
                            out=yv, in0=yv,
                            in1=gsl.unsqueeze(1).broadcast_to(
                                [128, WS, Wn * WS]),
                            op=ALU.mult)

            def ph_final(b):
                """Final channel matmul; GN5 sums via scalar accumulation."""
                S = st[b]
                y1 = S["y1"]
                ot = [pf32.tile([128, NPIX], F32, tag="f32",
                                name=f"ot{b}_{i}") for i in range(2)]
                S["ot"] = ot
                s15 = [sm.tile([128, 4], F32, tag="s15", bufs=2,
                               name=f"s15_{b}_{i}") for i in range(2)]
                s25 = [sm.tile([128, 2], F32, tag="s25", bufs=2,
                               name=f"s25_{b}_{i}") for i in range(2)]
                for m in range(2):
                    for nq in range(4):
                        pacc = psp.tile([128, 1024], F32, tag="acc", bufs=3,
                                        name=f"pcf_{b}_{m}_{nq}",
                                        uniquify=True)
                        for ni in range(2):
                            n = nq * 2 + ni
                            for kc in range(4):
                                nc.tensor.matmul(
                                    pacc[:, ni * 512:(ni + 1) * 512],
                                    S["wds"][kc][:, m * 128:(m + 1) * 128],
                                    y1[kc][:, bass.ts(n, 512)],
                                    start=(kc == 0), stop=(kc == 3))
                        nsl = bass.ts(nq, 1024)
                        nc.scalar.activation(
                            out=ot[m][:, nsl], in_=pacc, func=AF.Identity,
                            bias=S["cmt"][m], scale=1.0,
                            accum_out=s15[m][:, nq:nq + 1])
                        if nq % 2 == 1:
                            sqd = sm.tile([128, 2048], BF16, tag="sqd",
                                          bufs=1, name=f"sqd5_{b}_{m}_{nq}",
                                          uniquify=True)
                            nc.scalar.activation(
                                out=sqd, in_=ot[m][:, bass.ts(nq // 2, 2048)],
                                func=AF.Square, scale=1.0,
                                accum_out=s25[m][:, nq // 2:nq // 2 + 1])
                S["s15"], S["s25"] = s15, s25

            def ph_out(b):
                """GN5 + residual + store."""
                S = st[b]
                mv5 = []
                for c in range(2):
                    mv = sm.tile([128, 2], F32, tag="mv5", bufs=2,
                                 name=f"mv5_{b}_{c}")
                    nc.vector.tensor_reduce(out=mv[:, 0:1], in_=S["s15"][c],
                                            axis=AX.X, op=ALU.add)
                    nc.vector.tensor_reduce(out=mv[:, 1:2], in_=S["s25"][c],
                                            axis=AX.X, op=ALU.add)
                    nc.vector.tensor_scalar(
                        out=mv, in0=mv, scalar1=1.0 / NPIX, scalar2=None,
                        op0=ALU.mult)
                    mv5.append(mv)
                sc5 = gn_scale_bias(mv5, gm1_t, rep1_t, 32, "gn5",
                                    raw_ex2=True)
                ov = out_d[b].rearrange("c h w -> c (h w)")
                hsv = hs[b].rearrange("c h w -> c (h w)")
                ot = S["ot"]
                for c in range(2):
                    for q in range(4):
                        qsl = bass.ts(q, NPIX // 4)
                        xrq = pxr.tile([128, NPIX // 4], F32, tag="xr",
                                       name=f"xr{b}_{c}_{q}", uniquify=True)
                        nc.sync.dma_start(
                            out=xrq, in_=hsv[c * 128:(c + 1) * 128, qsl])
                        if q % 2 == 0:
                            nc.gpsimd.tensor_scalar(
                                out=ot[c][:, qsl], in0=ot[c][:, qsl],
                                scalar1=sc5[c][:, 0:1], scalar2=sc5[c][:, 1:2],
                                op0=ALU.mult, op1=ALU.add)
                            nc.vector.tensor_tensor(out=xrq,
                                                    in0=ot[c][:, qsl],
                                                    in1=xrq, op=ALU.add)
                        else:
                            nc.vector.tensor_scalar(
                                out=ot[c][:, qsl], in0=ot[c][:, qsl],
                                scalar1=sc5[c][:, 0:1], scalar2=sc5[c][:, 1:2],
                                op0=ALU.mult, op1=ALU.add)
                            nc.gpsimd.tensor_tensor(out=xrq,
                                                    in0=ot[c][:, qsl],
                                                    in1=xrq, op=ALU.add)
                        nc.sync.dma_start(
                            out=ov[c * 128:(c + 1) * 128, qsl],
                            in_=xrq)

            # ------------------------------------------------ emission
            def scoped(name, fn, *a):
                s, _ = nc.enter_named_scope(name, False)
                fn(*a)
                nc.leave_named_scope(name, s, False)

            scoped("ld_0", ph_load, 0)
            scoped("conv0_0", ph_conv0, 0)
            scoped("ld_1", ph_load, 1)
            scoped("conv1_0", ph_conv1, 0, (0, 1, 2, 3))
            scoped("conv0_1", ph_conv0, 1)
            scoped("attn_0", ph_attn, 0)
            scoped("conv1_1a", ph_conv1, 1, (0,))
            scoped("final_0", ph_final, 0)
            scoped("conv1_1b", ph_conv1, 1, (1, 2, 3))
            scoped("out_0", ph_out, 0)
            scoped("attn_1", ph_attn, 1)
            scoped("final_1", ph_final, 1)
            scoped("out_1", ph_out, 1)

    nc.compile()
    return nc


# ---------------------------------------------------------------- entry

_CACHE = {}


def _get_nc(sim_safe=False):
    key = bool(sim_safe)
    if key not in _CACHE:
        _CACHE[key] = build_nc(sim_safe=key)
    return _CACHE[key]


def make_in_maps(inputs):
    hs_full = np.ascontiguousarray(inputs["hidden_state"], dtype=np.float32)
    wd = _host_weights(
        np.asarray(inputs["w0"], np.float32), np.asarray(inputs["b0"], np.float32),
        np.asarray(inputs["w1"], np.float32), np.asarray(inputs["b1"], np.float32),
        np.asarray(inputs["w2"], np.float32), np.asarray(inputs["b2"], np.float32),
        np.asarray(inputs["w3"], np.float32), np.asarray(inputs["b3"], np.float32),
        np.asarray(inputs["weight"], np.float32))
    cm = _host_consts()
    cpack, bpack = _pack_consts(wd, cm)
    assert cpack.shape[1] == NCF, cpack.shape
    assert bpack.shape[1] == NBF, bpack.shape
    shared = {"cpack": cpack, "bpack": bpack}
    in_maps = []
    for i in range(NCORES):
        m = dict(shared)
        m["hs"] = np.ascontiguousarray(hs_full[i * BPC:(i + 1) * BPC])
        m["hsb"] = m["hs"].astype(ml_dtypes.bfloat16)
        in_maps.append(m)
    return in_maps


def kernel(**inputs):
    from concourse import bass_utils
    nc = _get_nc(sim_safe=False)
    in_maps = make_in_maps(inputs)
    res = bass_utils.run_bass_kernel_spmd(nc, in_maps,
                                          core_ids=list(range(NCORES)))
    out = np.concatenate([res.results[i]["out"] for i in range(NCORES)], axis=0)
    return out.astype(np.float32)


# revision 48
# speedup vs baseline: 1.4560x; 1.0394x over previous
"""Trainium2 Bass kernel for nn_Block_16544214024520 (dense_cnn).

Data-parallel over batch: 16 samples -> 2 per NeuronCore x 8 cores.
All parameters replicated. Per-sample layout: channels on partitions
(256 = 2 chunks of 128), pixels (64x64 = 4096) on the free dim.

Reference pipeline (per sample):
  gn(32) -> 1x1 conv(256->256)+silu -> gn(16) -> 3x3 grouped conv
  (g=4, 256->512)+silu -> gn(2) -> window-mean(8x8) -> radix amax ->
  1x1 g-conv(256->64)+silu -> gn(8) -> 1x1 g-conv(64->512) ->
  softmax over radix(2) -> gated combine -> channel matmul(256->512?no 256)
  -> gn(32) -> +residual

Optimizations over the straightforward version:
  * conv1 (3x3 grouped) runs 5 matmuls per row-tile instead of 18
    half-width ones: (ky=0,ky=1) taps pair into K=128 matmuls via a
    row-shifted duplicate of the padded input (SBUF->SBUF DMA), and
    (ky=2,kx=0,1) pair via a col-shifted duplicate; one K=64 single
    remains. Each weight block is reused across 4 row-tiles.
  * conv0/conv1/final accumulate into [128,1024] PSUM tiles (2 banks)
    -> half the scalar-engine evacuations.
  * GN3/GN5 statistics come from the scalar engine: means accumulate
    for free on the evacuation (accum_out), E[x^2] via a Square pass,
    replacing ~70us of DVE bn_stats.
  * radix softmax gating is a pure bf16 multiply: since a0+a1==1, the
    -mean3 shift const-folds into the final-evacuation bias.
  * window-pool partials in one tensor_reduce per group; per-group
    conv3->sigmoid->gate pipeline so gating starts early.
  * residual is loaded just-in-time in [128,1024] quarters; GN5 apply
    and the residual add alternate between gpsimd and DVE.
  * two samples emitted with a skewed, split-conv1 schedule so the
    second sample's conv1 fills the first sample's attention phase.
"""

import os
import sys

for _p in ("/opt/trn_rl_repo", "/opt/pypackages"):
    if _p not in sys.path:
        sys.path.append(_p)

import ml_dtypes
import numpy as np

import concourse.bass as bass  # noqa: F401
import concourse.mybir as mybir
import concourse.tile as tile
from concourse import bacc
from concourse.masks import make_identity

F32 = mybir.dt.float32
BF16 = mybir.dt.bfloat16
AF = mybir.ActivationFunctionType
ALU = mybir.AluOpType
AX = mybir.AxisListType

NCORES = 8
BPC = 2          # samples per core
C = 256          # channels
H = W = 64
NPIX = H * W     # 4096
PADW = W + 2     # 66
Hn = Wn = 8      # window grid
WS = 8           # window size
EPS = 1e-5
NT = 8           # n-tiles of 512 pixels (8 rows of 64)
XGROWS = 65      # rows in the shifted-dup conv1 input buffer


# ---------------------------------------------------------------- host prep

def _host_consts():
    """Constant matrices shared by all cores (built once)."""
    c = {}
    # GN over 256 channels, 32 groups of 8 (GN1/GN5)
    gm1 = np.zeros((2, 128, 32), np.float32)
    rep1 = np.zeros((2, 128, 128), np.float32)
    for ch in range(2):
        for k in range(128):
            g = (128 * ch + k) // 8
            gm1[ch, k, g] = 1.0 / 8.0
        for m in range(128):
            rep1[ch, (128 * ch + m) // 8 % 128, m] = 1.0
    c["gm1"] = gm1
    c["rep1"] = rep1
    # GN2: 16 groups of 16 over 256 channels
    gm2 = np.zeros((2, 128, 16), np.float32)
    rep2 = np.zeros((2, 128, 128), np.float32)
    for ch in range(2):
        for k in range(128):
            gm2[ch, k, (128 * ch + k) // 16] = 1.0 / 16.0
        for m in range(128):
            rep2[ch, (128 * ch + m) // 16, m] = 1.0
    c["gm2"] = gm2
    c["rep2"] = rep2
    # GN3 over 512 channels, 2 groups of 256 (chunks 0,1 -> g0; 2,3 -> g1)
    g3 = np.zeros((4, 128, 2), np.float32)
    r3 = np.zeros((4, 128, 128), np.float32)
    for mc in range(4):
        g3[mc, :, mc // 2] = 1.0 / 256.0
        r3[mc, mc // 2, :] = 1.0
    c["g3"] = g3
    c["r3"] = r3
    # GN4 over 64 channels, 8 groups of 8
    g4 = np.zeros((128, 8), np.float32)
    for k in range(64):
        g4[k, k // 8] = 1.0 / 8.0
    r4 = np.zeros((128, 64), np.float32)
    for m in range(64):
        r4[m // 8, m] = 1.0
    c["g4"] = g4
    c["r4"] = r4
    return c


def _host_weights(w0, b0, w1, b1, w2, b2, w3, b3, weight):
    """Rearrange torch-layout conv weights into matmul lhsT tensors."""
    d = {}
    # conv0: out[o,p] = sum_i w0[o,i] x[i,p]  -> lhsT[i,o]
    d["w0T"] = np.ascontiguousarray(w0[:, :, 0, 0].T).astype(
        ml_dtypes.bfloat16)  # [256,256]
    d["b0c"] = np.ascontiguousarray(b0.reshape(C, 1)).astype(np.float32)
    # conv1: grouped 3x3, groups=4 (in 64 -> out 128 each).
    # Row-pair lhsT per (g, dx): [128,128] rows 0:64 = ky=0, rows 64:128 =
    # ky=1 (paired via the row-shifted dup buffer xg).
    # Col-pair lhsT per g: rows 0:64 = (ky=2, kx=0), rows 64:128 =
    # (ky=2, kx=1) (paired via the col-shifted dup buffer xh).
    # Last single per g: rows 0:64 = (ky=2, kx=2).
    w1p = np.zeros((4, 3, 128, 128), np.float32)
    w1c = np.zeros((4, 128, 128), np.float32)
    w1e = np.zeros((4, 128, 128), np.float32)
    for g in range(4):
        for dx in range(3):
            w1p[g, dx, 0:64, :] = w1[g * 128:(g + 1) * 128, :, 0, dx].T
            w1p[g, dx, 64:128, :] = w1[g * 128:(g + 1) * 128, :, 1, dx].T
        w1c[g, 0:64, :] = w1[g * 128:(g + 1) * 128, :, 2, 0].T
        w1c[g, 64:128, :] = w1[g * 128:(g + 1) * 128, :, 2, 1].T
        w1e[g, 0:64, :] = w1[g * 128:(g + 1) * 128, :, 2, 2].T
    d["w1p"] = w1p.astype(ml_dtypes.bfloat16)
    d["w1c"] = w1c.astype(ml_dtypes.bfloat16)
    d["w1e"] = w1e.astype(ml_dtypes.bfloat16)
    d["b1c"] = np.ascontiguousarray(b1.reshape(2 * C, 1)).astype(np.float32)
    # conv2: groups=2 (in 128 -> out 32)
    w2t = np.zeros((2, 128, 32), np.float32)
    for g in range(2):
        w2t[g] = w2[g * 32:(g + 1) * 32, :, 0, 0].T
    d["w2t"] = w2t
    d["b2c"] = np.ascontiguousarray(b2.reshape(64, 1)).astype(np.float32)
    # conv3: groups=2 (in 32 -> out 256); K padded to 128 with zero rows.
    w3t = np.zeros((4, 128, 128), np.float32)
    for g in range(4):
        src = w3[g * 128:(g + 1) * 128, :, 0, 0]      # [128, 32]
        r0 = 0 if g < 2 else 32
        w3t[g, r0:r0 + 32, :] = src.T
    d["w3t"] = w3t
    # final einsum: out[c,p] = sum_C weight[C,c] z[C,p], z[C] = zint[2C]+zint[2C+1]
    # fold the radix pair-sum by duplicating rows: wdup[c512, c] = weight[c512//2, c]
    wdup = np.repeat(weight.astype(np.float32), 2, axis=0)   # [512, 256]
    d["wdupT"] = np.ascontiguousarray(wdup).astype(ml_dtypes.bfloat16)
    return d


def _pack_consts(wd, cm):
    """Pack all fp32 constants into one [128, F] tensor and all bf16
    weights into another, so startup needs only two DMAs."""
    fcols = []   # list of [128, n] fp32 blocks
    def addf(x):
        x = np.asarray(x, np.float32)
        assert x.shape[0] == 128
        fcols.append(x.reshape(128, -1))
    for c in range(2):
        addf(cm["gm1"][c]); addf(cm["rep1"][c])
        addf(cm["gm2"][c]); addf(cm["rep2"][c])
    for g in range(4):
        addf(cm["g3"][g]); addf(cm["r3"][g])
    addf(cm["g4"]); addf(cm["r4"])
    b0 = wd["b0c"].reshape(2, 128, 1)
    addf(b0[0]); addf(b0[1])
    b1 = wd["b1c"].reshape(4, 128, 1)
    for g in range(4):
        addf(b1[g])
    b2p = np.zeros((128, 1), np.float32)
    b2p[0:64] = wd["b2c"]
    addf(b2p)
    addf(np.full((128, 1), EPS, np.float32))
    for g in range(2):
        addf(wd["w2t"][g])
    for g in range(4):
        addf(wd["w3t"][g])
    cpack = np.concatenate(fcols, axis=1)

    w0 = np.asarray(wd["w0T"])
    bcols = [w0[0:128], w0[128:256]]
    for g in range(4):
        for dx in range(3):
            bcols.append(np.asarray(wd["w1p"])[g, dx])
        bcols.append(np.asarray(wd["w1c"])[g])
        bcols.append(np.asarray(wd["w1e"])[g])
    wdp = np.asarray(wd["wdupT"])
    for k in range(4):
        bcols.append(wdp[k * 128:(k + 1) * 128])
    bpack = np.concatenate(bcols, axis=1).astype(ml_dtypes.bfloat16)
    return cpack, bpack


NCF = 32 + 128 + 16 + 128 + 32 + 128 + 16 + 128 + 4 * (2 + 128) \
    + 8 + 64 + 2 + 4 + 1 + 1 + 2 * 32 + 4 * 128
NBF = 256 * 2 + 4 * 5 * 128 + 4 * 256


# ---------------------------------------------------------------- builder

def build_nc(sim_safe: bool = False):
    nc = bacc.Bacc("TRN2", target_bir_lowering=False, debug=False,
                   num_devices=NCORES)

    def din(name, shape, dt=F32):
        return nc.dram_tensor(name, list(shape), dt, kind="ExternalInput").ap()

    hs = din("hs", (BPC, C, H, W))
    hsb = din("hsb", (BPC, C, H, W), BF16)
    cpack_d = din("cpack", (128, NCF))
    bpack_d = din("bpack", (128, NBF), BF16)

    out_d = nc.dram_tensor("out", [BPC, C, H, W], F32, kind="ExternalOutput").ap()

    with tile.TileContext(nc) as tc:
        with tc.tile_pool(name="consts", bufs=1) as cst, \
             tc.tile_pool(name="b16", bufs=8) as pb16, \
             tc.tile_pool(name="xq", bufs=2) as pxq, \
             tc.tile_pool(name="xg", bufs=2) as pxg, \
             tc.tile_pool(name="xh", bufs=2) as pxh, \
             tc.tile_pool(name="f32", bufs=2) as pf32, \
             tc.tile_pool(name="xr", bufs=3) as pxr, \
             tc.tile_pool(name="small", bufs=2) as sm, \
             tc.tile_pool(name="psum", bufs=2, space="PSUM") as psp:

            # ---- load constants / weights (two packed DMAs) ----
            cpk = cst.tile([128, NCF], F32, name="cpk")
            nc.sync.dma_start(out=cpk, in_=cpack_d)
            bpk = cst.tile([128, NBF], BF16, name="bpk")
            nc.sync.dma_start(out=bpk, in_=bpack_d)

            class _Cur:
                def __init__(self):
                    self.o = 0
            _cf, _cb = _Cur(), _Cur()

            def fsl(n):
                s = cpk[:, _cf.o:_cf.o + n]
                _cf.o += n
                return s

            def bsl(n):
                s = bpk[:, _cb.o:_cb.o + n]
                _cb.o += n
                return s

            gm1_t, rep1_t, gm2_t, rep2_t = [], [], [], []
            for c in range(2):
                gm1_t.append(fsl(32)); rep1_t.append(fsl(128))
                gm2_t.append(fsl(16)); rep2_t.append(fsl(128))
            g3_t, r3_t = [], []
            for g in range(4):
                g3_t.append(fsl(2)); r3_t.append(fsl(128))
            g4_t = fsl(8); r4_t = fsl(64)
            b0_t = [fsl(1) for _ in range(2)]
            b1_t = [fsl(1) for _ in range(4)]
            b2_t = fsl(1)
            eps_t = fsl(1)
            w2_t = [fsl(32) for _ in range(2)]
            w3_t = [fsl(128) for _ in range(4)]
            assert _cf.o == NCF
            w0_t = [bsl(256) for _ in range(2)]
            w1p_t = [[None] * 3 for _ in range(4)]
            w1c_t = [None] * 4
            w1e_t = [None] * 4
            for g in range(4):
                for dx in range(3):
                    w1p_t[g][dx] = bsl(128)
                w1c_t[g] = bsl(128)
                w1e_t[g] = bsl(128)
            wd_t = [bsl(256) for _ in range(4)]
            assert _cb.o == NBF
            ident = cst.tile([128, 128], F32, name="ident")
            make_identity(nc, ident)

            # ------------------------------------------------ helpers
            def silu_evac(out_ap, psum_ap, bias_ap, tag, accum_out=None):
                """out = silu(psum + bias); fused on HW, 2-op in CoreSim."""
                if not sim_safe:
                    nc.scalar.activation(out=out_ap, in_=psum_ap, func=AF.Silu,
                                         bias=bias_ap, scale=1.0,
                                         accum_out=accum_out)
                    return
                if True:
                    ff = psum_ap.free_size()
                    pp = psum_ap.partition_size()
                    sgf = sm.tile([128, 1024], F32, tag="sg", bufs=1,
                                  name=f"sg_{tag}", uniquify=True)
                    sgt = sgf[0:pp, 0:ff]
                    nc.scalar.activation(out=sgt, in_=psum_ap, func=AF.Sigmoid,
                                         bias=bias_ap, scale=1.0)
                    nc.vector.scalar_tensor_tensor(
                        out=out_ap, in0=psum_ap, scalar=bias_ap, in1=sgt,
                        op0=ALU.add, op1=ALU.mult)
                    if accum_out is not None:
                        nc.scalar.activation(out=sgt, in_=out_ap,
                                             func=AF.Identity, scale=1.0,
                                             accum_out=accum_out)

            def gn_scale_bias(mvs, gmat_list, rmat_list, ngroups, tag,
                              ncols=2, raw_ex2=False):
                """Per-channel (scale, bias) tiles for a group norm.

                mvs: list of [128, 2] SBUF tiles of per-channel (mean, var).
                Returns list of [128, ncols] tiles (col0 = rstd,
                col1 = -mean*rstd, col2 = -mean) replicated back to channels.
                """
                nchunk = len(mvs)
                if raw_ex2:
                    # mvs are already [128, 2] = (mean, E[x^2]) tiles
                    rstats = mvs
                else:
                    rstats = []
                    for ci, mv in enumerate(mvs):
                        r = sm.tile([128, 2], F32, tag=f"r_{tag}",
                                    bufs=2 * nchunk)
                        nc.vector.tensor_copy(out=r[:, 0:1], in_=mv[:, 0:1])
                        nc.vector.scalar_tensor_tensor(
                            out=r[:, 1:2], in0=mv[:, 0:1], scalar=mv[:, 0:1],
                            in1=mv[:, 1:2], op0=ALU.mult, op1=ALU.add)
                        rstats.append(r)
                pg = psp.tile([128, 2], F32, tag="gn_ps", bufs=1)
                for ci in range(nchunk):
                    nc.tensor.matmul(pg[0:ngroups, :], gmat_list[ci], rstats[ci],
                                     start=(ci == 0), stop=(ci == nchunk - 1))
                gt = sm.tile([128, 2], F32, tag=f"gt_{tag}", bufs=2)
                nc.vector.memset(gt, 0.0)
                nc.scalar.copy(out=gt[0:ngroups, :], in_=pg[0:ngroups, :])
                # -var = mean^2 - E[x^2]
                negv = sm.tile([128, 1], F32, tag=f"nv_{tag}", bufs=2)
                nc.vector.scalar_tensor_tensor(
                    out=negv[0:ngroups], in0=gt[0:ngroups, 0:1],
                    scalar=gt[0:ngroups, 0:1], in1=gt[0:ngroups, 1:2],
                    op0=ALU.mult, op1=ALU.subtract)
                sd = sm.tile([128, 1], F32, tag=f"sd_{tag}", bufs=2)
                nc.scalar.activation(out=sd[0:ngroups], in_=negv[0:ngroups],
                                     func=AF.Sqrt, bias=eps_t[0:ngroups],
                                     scale=-1.0)
                rstd = sm.tile([128, 1], F32, tag=f"rs_{tag}", bufs=2)
                nc.vector.reciprocal(out=rstd[0:ngroups], in_=sd[0:ngroups])
                stg = sm.tile([128, 3], F32, tag=f"st_{tag}", bufs=2)
                nc.vector.memset(stg, 0.0)
                nc.vector.tensor_copy(out=stg[0:ngroups, 0:1], in_=rstd[0:ngroups])
                nc.vector.tensor_scalar(
                    out=stg[0:ngroups, 1:2], in0=gt[0:ngroups, 0:1],
                    scalar1=rstd[0:ngroups], scalar2=-1.0,
                    op0=ALU.mult, op1=ALU.mult)
                if ncols == 3:
                    nc.vector.tensor_scalar(
                        out=stg[0:ngroups, 2:3], in0=gt[0:ngroups, 0:1],
                        scalar1=-1.0, scalar2=None, op0=ALU.mult)
                scs = []
                for ci, rmat in enumerate(rmat_list):
                    mm = rmat.shape[-1]
                    pr = psp.tile([128, 3], F32, tag="gn_ps", bufs=1)
                    nc.tensor.matmul(pr[0:mm, 0:ncols], rmat,
                                     stg[:, 0:ncols], start=True, stop=True)
                    sc = sm.tile([128, 3], F32, tag=f"sc_{tag}",
                                 bufs=2 * nchunk)
                    nc.scalar.copy(out=sc[0:mm, 0:ncols], in_=pr[0:mm, 0:ncols])
                    scs.append(sc)
                return scs

            st = [dict() for _ in range(BPC)]

            # ------------------------------------------------ phases
            def ph_load(b):
                """Load input, GN1 stats, fold GN1 into conv0 weights."""
                S = st[b]
                hsbv = hsb[b].rearrange("c h w -> c (h w)")
                S["xw"] = [pb16.tile([128, NPIX], BF16, tag="b16",
                                     name=f"xw{b}_{i}") for i in range(2)]
                bst1 = [sm.tile([128, NT, 6], F32, tag="bst1", bufs=2,
                                name=f"bst1_{b}_{i}") for i in range(2)]
                for c in range(2):
                    nc.sync.dma_start(out=S["xw"][c],
                                      in_=hsbv[c * 128:(c + 1) * 128, :])
                    for n in range(NT):
                        nc.vector.bn_stats(out=bst1[c][:, n, :],
                                           in_=S["xw"][c][:, bass.ts(n, 512)])
                mv1 = []
                for c in range(2):
                    mv = sm.tile([128, 2], F32, tag="mv1", bufs=2,
                                 name=f"mv1_{b}_{c}")
                    nc.vector.bn_aggr(out=mv, in_=bst1[c])
                    mv1.append(mv)
                sc1 = gn_scale_bias(mv1, gm1_t, rep1_t, 32, "gn1")
                # fold GN1 into conv0 weights
                w0s = [sm.tile([128, 256], BF16, tag="w0s", bufs=2,
                               name=f"w0s{b}_{i}") for i in range(2)]
                t1b = [sm.tile([128, 1], BF16, tag="t1b", bufs=2,
                               name=f"t1b{b}_{i}") for i in range(2)]
                for c in range(2):
                    nc.vector.tensor_scalar_mul(out=w0s[c], in0=w0_t[c],
                                                scalar1=sc1[c][:, 0:1])
                    nc.vector.tensor_copy(out=t1b[c], in_=sc1[c][:, 1:2])
                b0p = [sm.tile([128, 1], F32, tag="b0p", bufs=2,
                               name=f"b0p{b}_{i}") for i in range(2)]
                for m in range(2):
                    pb = psp.tile([128, 1], F32, tag="gn_ps", bufs=1)
                    for kc in range(2):
                        nc.tensor.matmul(
                            pb,
                            w0s[kc][:, m * 128:(m + 1) * 128],
                            t1b[kc],
                            start=(kc == 0), stop=(kc == 1))
                    nc.scalar.activation(out=b0p[m], in_=pb,
                                         func=AF.Identity, bias=b0_t[m],
                                         scale=1.0)
                S["w0s"] = w0s
                S["b0p"] = b0p

            def ph_conv0(b):
                """conv0 (1x1)+silu straight into padded conv1 input; GN2
                stats + in-place apply; build shifted-dup buffers by DMA."""
                S = st[b]
                xq = [pxq.tile([128, PADW, PADW], BF16, tag="xq",
                               name=f"xq{b}_{i}") for i in range(2)]
                S["xq"] = xq
                for c in range(2):
                    xpf = xq[c]
                    nc.gpsimd.memset(xpf[:, 0:1, :], 0.0)
                    nc.gpsimd.memset(xpf[:, PADW - 1:PADW, :], 0.0)
                    nc.gpsimd.memset(xpf[:, 1:PADW - 1, 0:1], 0.0)
                    nc.gpsimd.memset(xpf[:, 1:PADW - 1, PADW - 1:PADW], 0.0)
                y0 = [pb16.tile([128, NPIX], BF16, tag="b16",
                                name=f"y0{b}_{i}") for i in range(2)]
                bst2 = [sm.tile([128, NT, 6], F32, tag="bst2", bufs=2,
                                name=f"bst2_{b}_{i}") for i in range(2)]
                for m in range(2):
                    for nq in range(4):
                        pacc = psp.tile([128, 1024], F32, tag="acc", bufs=3,
                                        name=f"pc0_{b}_{m}_{nq}",
                                        uniquify=True)
                        for ni in range(2):
                            n = nq * 2 + ni
                            for kc in range(2):
                                nc.tensor.matmul(
                                    pacc[:, ni * 512:(ni + 1) * 512],
                                    S["w0s"][kc][:, m * 128:(m + 1) * 128],
                                    S["xw"][kc][:, bass.ts(n, 512)],
                                    start=(kc == 0), stop=(kc == 1))
                        nsl = bass.ts(nq, 1024)
                        silu_evac(y0[m][:, nsl], pacc, S["b0p"][m],
                                  f"c0_{b}")
                        for ni in range(2):
                            n = nq * 2 + ni
                            nc.vector.bn_stats(out=bst2[m][:, n, :],
                                               in_=y0[m][:, bass.ts(n, 512)])
                mv2 = []
                for c in range(2):
                    mv = sm.tile([128, 2], F32, tag="mv2", bufs=2,
                                 name=f"mv2_{b}_{c}")
                    nc.vector.bn_aggr(out=mv, in_=bst2[c])
                    mv2.append(mv)
                sc2 = gn_scale_bias(mv2, gm2_t, rep2_t, 16, "gn2")
                for c in range(2):
                    nc.gpsimd.tensor_scalar(
                        out=xq[c][:, 1:65, 1:65],
                        in0=y0[c].rearrange("p (h w) -> p h w", h=H),
                        scalar1=sc2[c][:, 0:1], scalar2=sc2[c][:, 1:2],
                        op0=ALU.mult, op1=ALU.add)
                # shifted-dup buffers for conv1 tap pairing:
                # parts 0:64 <- xq rows 0..64 (offset r*66 holds image row r-1)
                # parts 64:128 <- xq rows 1..65 (offset r*66 holds image row r)
                xg = [pxg.tile([128, XGROWS, PADW], BF16, tag="xg",
                               name=f"xg{b}_{g}") for g in range(4)]
                xh = [pxh.tile([128, XGROWS, PADW], BF16, tag="xh",
                               name=f"xh{b}_{g}") for g in range(4)]
                S["xg"] = xg
                S["xh"] = xh
                for g in range(4):
                    kc, blk = g // 2, g % 2
                    src = xq[kc]
                    p0 = blk * 64
                    nc.sync.dma_start(
                        out=xg[g][0:64, :, :],
                        in_=src[p0:p0 + 64, 0:XGROWS, :])
                    nc.sync.dma_start(
                        out=xg[g][64:128, :, :],
                        in_=src[p0:p0 + 64, 1:1 + XGROWS, :])
                    nc.sync.dma_start(
                        out=xh[g][0:64, :, :],
                        in_=src[p0:p0 + 64, 1:1 + XGROWS, :])
                    nc.sync.dma_start(
                        out=xh[g][64:128, :, 0:PADW - 1],
                        in_=src[p0:p0 + 64, 1:1 + XGROWS, 1:PADW])

            def ph_conv1(b, gs):
                """conv1 (3x3 grouped, tap-paired) + silu -> y1 for groups
                in gs; GN3 stats and window-pool partials in-loop."""
                S = st[b]
                if 0 in gs:
                    S["y1"] = [pb16.tile([128, NPIX], BF16, tag="b16",
                                         name=f"y1{b}_{g}") for g in range(4)]
                    S["s13"] = [sm.tile([128, 4], F32, tag="s13", bufs=4,
                                        name=f"s13_{b}_{g}") for g in range(4)]
                    S["s23"] = [sm.tile([128, 2], F32, tag="s23", bufs=4,
                                        name=f"s23_{b}_{g}") for g in range(4)]
                    S["amT"] = sm.tile([64, 256], F32, tag="amT", bufs=1,
                                       name=f"amT{b}")
                y1, s13, s23, amT = S["y1"], S["s13"], S["s23"], S["amT"]
                for g in gs:
                    pa_g = sm.tile([128, NT * 64], BF16, tag="pa", bufs=2,
                                   name=f"pa{b}_{g}")
                    xgv = S["xg"][g]
                    xhv = S["xh"][g]
                    for np2 in range(2):
                        paccs = [psp.tile([128, 1024], F32, tag="acc", bufs=3,
                                          name=f"pc1_{b}_{g}_{np2}_{t}",
                                          uniquify=True) for t in range(2)]
                        # each weight block is loaded once and streamed over
                        # 4 row-tiles (2 psum tiles x 2 halves)
                        for dx in range(3):
                            for t in range(2):
                                for ni in range(2):
                                    n = (np2 * 2 + t) * 2 + ni
                                    r0 = n * WS
                                    nc.tensor.matmul(
                                        paccs[t][:, ni * 512:(ni + 1) * 512],
                                        w1p_t[g][dx],
                                        xgv[:, r0:r0 + 8, dx:dx + 64],
                                        start=(dx == 0), stop=False)
                        for t in range(2):
                            for ni in range(2):
                                n = (np2 * 2 + t) * 2 + ni
                                r0 = n * WS
                                nc.tensor.matmul(
                                    paccs[t][:, ni * 512:(ni + 1) * 512],
                                    w1c_t[g],
                                    xhv[:, r0 + 1:r0 + 9, 0:64],
                                    start=False, stop=False)
                        for t in range(2):
                            for ni in range(2):
                                n = (np2 * 2 + t) * 2 + ni
                                r0 = n * WS
                                nc.tensor.matmul(
                                    paccs[t][:, ni * 512:(ni + 1) * 512],
                                    w1e_t[g][0:64, :],
                                    xhv[0:64, r0 + 1:r0 + 9, 2:66],
                                    start=False, stop=True)
                        for t in range(2):
                            npair = np2 * 2 + t
                            nsl = bass.ts(npair, 1024)
                            silu_evac(y1[g][:, nsl], paccs[t], b1_t[g],
                                      f"c1_{b}",
                                      accum_out=s13[g][:, npair:npair + 1])
                        sqd = sm.tile([128, 2048], BF16, tag="sqd",
                                      bufs=1, name=f"sqd3_{b}_{g}_{np2}",
                                      uniquify=True)
                        nc.scalar.activation(
                            out=sqd, in_=y1[g][:, bass.ts(np2, 2048)],
                            func=AF.Square, scale=1.0,
                            accum_out=s23[g][:, np2:np2 + 1])
                    # window-pool partials in one reduce per group
                    with nc.allow_low_precision(reason="bf16 pool partials"):
                        nc.vector.tensor_reduce(
                            out=pa_g,
                            in_=y1[g].rearrange("p (a w2) -> p a w2", w2=WS),
                            axis=AX.X, op=ALU.add)
                    # finish this group's window means + transpose + radix max
                    pooled = sm.tile([128, Hn, Wn], F32, tag="pooled", bufs=2,
                                     name=f"pooled{b}_{g}", uniquify=True)
                    pav = pa_g.rearrange("p (hn h2 wn) -> p hn wn h2",
                                         hn=Hn, h2=WS)
                    nc.vector.tensor_reduce(out=pooled, in_=pav,
                                            axis=AX.X, op=ALU.add)
                    ptp = psp.tile([64, 128], F32, tag="tp", bufs=1)
                    nc.tensor.transpose(
                        ptp, pooled.rearrange("p a b -> p (a b)"), ident)
                    pooledT = sm.tile([64, 128], F32, tag="pooledT", bufs=2,
                                      name=f"pooledT{b}_{g}", uniquify=True)
                    nc.scalar.copy(out=pooledT, in_=ptp)
                    pv = pooledT.rearrange("p (a b) -> p a b", b=2)
                    nc.vector.tensor_tensor(
                        out=amT[:, g * 64:(g + 1) * 64],
                        in0=pv[:, :, 0], in1=pv[:, :, 1], op=ALU.max)
                if 3 not in gs:
                    return
                mv3 = []
                for g in range(4):
                    mv = sm.tile([128, 2], F32, tag="mv3", bufs=4,
                                 name=f"mv3_{b}_{g}")
                    nc.vector.tensor_reduce(out=mv[:, 0:1], in_=s13[g],
                                            axis=AX.X, op=ALU.add)
                    nc.vector.tensor_reduce(out=mv[:, 1:2], in_=s23[g],
                                            axis=AX.X, op=ALU.add)
                    nc.vector.tensor_scalar(
                        out=mv, in0=mv, scalar1=1.0 / NPIX, scalar2=None,
                        op0=ALU.mult)
                    mv3.append(mv)
                sc3 = gn_scale_bias(mv3, g3_t, r3_t, 2, "gn3", ncols=3,
                                    raw_ex2=True)
                S["sc3"] = sc3
                # fold GN3 scale into the final matmul weights
                wds = [sm.tile([128, 256], BF16, tag="wds", bufs=4,
                               name=f"wds{b}_{kc}") for kc in range(4)]
                t3b = [sm.tile([128, 1], BF16, tag="t3b", bufs=4,
                               name=f"t3b{b}_{kc}") for kc in range(4)]
                for kc in range(4):
                    nc.vector.tensor_scalar_mul(
                        out=wds[kc], in0=wd_t[kc],
                        scalar1=sc3[kc][:, 0:1])
                    # 0.5: the wdup row-duplication would count m3 twice
                    nc.vector.tensor_scalar(
                        out=t3b[kc], in0=sc3[kc][:, 2:3], scalar1=0.5,
                        scalar2=None, op0=ALU.mult)
                # const-fold: cm[co] = sum_C wds[C,co] * (-m3[C]); becomes
                # the final-evac bias (valid because a0 + a1 == 1).
                cmt = [sm.tile([128, 1], F32, tag="cmt", bufs=2,
                               name=f"cmt{b}_{m}") for m in range(2)]
                for m in range(2):
                    pcm = psp.tile([128, 1], F32, tag="gn_ps", bufs=1)
                    for kc in range(4):
                        nc.tensor.matmul(
                            pcm, wds[kc][:, m * 128:(m + 1) * 128], t3b[kc],
                            start=(kc == 0), stop=(kc == 3))
                    nc.scalar.copy(out=cmt[m], in_=pcm)
                S["cmt"] = cmt
                S["wds"] = wds

            def ph_attn(b):
                """Window mean finish, radix amax, conv2+GN4+conv3,
                softmax -> per-group gate tiles; also load the residual."""
                S = st[b]
                sc3 = S["sc3"]
                amT = S["amT"]
                am = [sm.tile([128, 64], F32, tag="am", bufs=2,
                              name=f"am{b}_{i}") for i in range(2)]
                s64 = [sm.tile([128, 1], F32, tag="s64", bufs=2,
                               name=f"s64_{b}_{i}") for i in range(2)]
                for c in range(2):
                    pta = psp.tile([128, 64], F32, tag="tp", bufs=1)
                    nc.tensor.transpose(pta, amT[:, c * 128:(c + 1) * 128],
                                        ident[0:64, 0:64])
                    nc.scalar.copy(out=am[c], in_=pta)
                    # normalize the pooled maxima: am = am*(s3/64) + t3
                    nc.vector.tensor_scalar(
                        out=s64[c], in0=sc3[2 * c][:, 0:1],
                        scalar1=1.0 / (WS * WS), scalar2=None, op0=ALU.mult)
                    nc.vector.tensor_scalar(
                        out=am[c], in0=am[c], scalar1=s64[c],
                        scalar2=sc3[2 * c][:, 1:2], op0=ALU.mult, op1=ALU.add)

                # ---- conv2 (1x1 g=2, 256->64) + silu ----
                p2 = psp.tile([128, 64], F32, tag="tp", bufs=1)
                for g in range(2):
                    nc.tensor.matmul(p2[g * 32:(g + 1) * 32, :], w2_t[g], am[g],
                                     start=True, stop=True)
                a2 = sm.tile([128, 64], F32, tag="a2", bufs=2)
                nc.vector.memset(a2, 0.0)
                silu_evac(a2[0:64, :], p2[0:64, :], b2_t[0:64], f"c2_{b}")

                # ---- GN4 -> a2n ----
                mv4pad = sm.tile([128, 2], F32, tag="mv4", bufs=2)
                nc.vector.memset(mv4pad, 0.0)
                bst4 = sm.tile([128, 1, 6], F32, tag="bst4", bufs=2)
                nc.vector.bn_stats(out=bst4[0:64], in_=a2[0:64].unsqueeze(1))
                nc.vector.bn_aggr(out=mv4pad[0:64], in_=bst4[0:64])
                sc4 = gn_scale_bias([mv4pad], [g4_t], [r4_t], 8, "gn4")[0]
                a2n = sm.tile([128, 64], F32, tag="a2n", bufs=2)
                nc.vector.memset(a2n, 0.0)
                nc.vector.tensor_scalar(
                    out=a2n[0:64], in0=a2[0:64],
                    scalar1=sc4[0:64, 0:1], scalar2=sc4[0:64, 1:2],
                    op0=ALU.mult, op1=ALU.add)

                # ---- conv3 (1x1 g=2, 64->512), b3 = 0; then softmax over
                # radix == sigmoid of pair difference; fully per-group so the
                # first gate tile is ready early ----
                grow = [sm.tile([128, Hn, Wn, WS], BF16, tag="grow", bufs=4,
                                name=f"grow{b}_{g}") for g in range(4)]
                for g in range(4):
                    p3 = psp.tile([128, 64], F32, tag="tp", bufs=1)
                    nc.tensor.matmul(p3, w3_t[g], a2n, start=True, stop=True)
                    a3 = sm.tile([128, 64], F32, tag="a3", bufs=2)
                    nc.scalar.copy(out=a3, in_=p3)
                    p3t = psp.tile([64, 128], F32, tag="tp", bufs=1)
                    nc.tensor.transpose(p3t, a3, ident)
                    a3Tg = sm.tile([64, 128], F32, tag="a3T", bufs=2,
                                   name=f"a3T{b}_{g}", uniquify=True)
                    nc.scalar.copy(out=a3Tg, in_=p3t)
                    a3v = a3Tg.rearrange("p (a b) -> p a b", b=2)
                    dTg = sm.tile([64, 64], F32, tag="dT", bufs=2,
                                  name=f"dT{b}_{g}", uniquify=True)
                    nc.vector.tensor_tensor(out=dTg, in0=a3v[:, :, 0],
                                            in1=a3v[:, :, 1], op=ALU.subtract)
                    sTg = sm.tile([64, 128], F32, tag="sT", bufs=2,
                                  name=f"sT{b}_{g}", uniquify=True)
                    sTv = sTg.rearrange("p (a b) -> p a b", b=2)
                    nc.scalar.activation(out=sTv[:, :, 0], in_=dTg,
                                         func=AF.Sigmoid, scale=1.0)
                    nc.scalar.activation(out=sTv[:, :, 1], in_=dTg,
                                         func=AF.Sigmoid, scale=-1.0)
                    pst = psp.tile([128, 64], F32, tag="tp", bufs=1)
                    nc.tensor.transpose(pst, sTg, ident[0:64, 0:64])
                    sintg = sm.tile([128, 64], F32, tag="sint", bufs=4,
                                    name=f"sint{b}_{g}")
                    nc.scalar.copy(out=sintg, in_=pst)
                    gv = sintg.rearrange("p (hn wn) -> p hn wn", hn=Hn)
                    nc.gpsimd.tensor_copy(
                        out=grow[g],
                        in_=gv.unsqueeze(3).broadcast_to([128, Hn, Wn, WS]))
                    # gate this group's y1 in place right away (pure multiply;
                    # the -m3 shift is const-folded into the final-evac bias)
                    y1 = S["y1"]
                    for hn in range(Hn):
                        gsl = grow[g][:, hn, :, :].rearrange("p a b -> p (a b)")
                        yv = y1[g][:, bass.ts(hn, 512)].rearrange(
                            "p (h2 x) -> p h2 x", h2=WS)
                        nc.vector.tensor_tensor(
                            out=yv, in0=yv,
                            in1=gsl.unsqueeze(1).broadcast_to(
                                [128, WS, Wn * WS]),
                            op=ALU.mult)

            def ph_final(b):
                """Final channel matmul; GN5 sums via scalar accumulation."""
                S = st[b]
                y1 = S["y1"]
                ot = [pf32.tile([128, NPIX], F32, tag="f32",
                                name=f"ot{b}_{i}") for i in range(2)]
                S["ot"] = ot
                s15 = [sm.tile([128, 4], F32, tag="s15", bufs=2,
                               name=f"s15_{b}_{i}") for i in range(2)]
                s25 = [sm.tile([128, 2], F32, tag="s25", bufs=2,
                               name=f"s25_{b}_{i}") for i in range(2)]
                for m in range(2):
                    for nq in range(4):
                        pacc = psp.tile([128, 1024], F32, tag="acc", bufs=3,
                                        name=f"pcf_{b}_{m}_{nq}",
                                        uniquify=True)
                        for ni in range(2):
                            n = nq * 2 + ni
                            for kc in range(4):
                                nc.tensor.matmul(
                                    pacc[:, ni * 512:(ni + 1) * 512],
                                    S["wds"][kc][:, m * 128:(m + 1) * 128],
                                    y1[kc][:, bass.ts(n, 512)],
                                    start=(kc == 0), stop=(kc == 3))
                        nsl = bass.ts(nq, 1024)
                        nc.scalar.activation(
                            out=ot[m][:, nsl], in_=pacc, func=AF.Identity,
                            bias=S["cmt"][m], scale=1.0,
                            accum_out=s15[m][:, nq:nq + 1])
                        if nq % 2 == 1:
                            sqd = sm.tile([128, 2048], BF16, tag="sqd",
                                          bufs=1, name=f"sqd5_{b}_{m}_{nq}",
                                          uniquify=True)
                            nc.scalar.activation(
                                out=sqd, in_=ot[m][:, bass.ts(nq // 2, 2048)],
                                func=AF.Square, scale=1.0,
                                accum_out=s25[m][:, nq // 2:nq // 2 + 1])
                S["s15"], S["s25"] = s15, s25

            def ph_out(b):
                """GN5 + residual + store."""
                S = st[b]
                mv5 = []
                for c in range(2):
                    mv = sm.tile([128, 2], F32, tag="mv5", bufs=2,
                                 name=f"mv5_{b}_{c}")
                    nc.vector.tensor_reduce(out=mv[:, 0:1], in_=S["s15"][c],
                                            axis=AX.X, op=ALU.add)
                    nc.vector.tensor_reduce(out=mv[:, 1:2], in_=S["s25"][c],
                                            axis=AX.X, op=ALU.add)
                    nc.vector.tensor_scalar(
                        out=mv, in0=mv, scalar1=1.0 / NPIX, scalar2=None,
                        op0=ALU.mult)
                    mv5.append(mv)
                sc5 = gn_scale_bias(mv5, gm1_t, rep1_t, 32, "gn5",
                                    raw_ex2=True)
                ov = out_d[b].rearrange("c h w -> c (h w)")
                hsv = hs[b].rearrange("c h w -> c (h w)")
                ot = S["ot"]
                for c in range(2):
                    for q in range(4):
                        qsl = bass.ts(q, NPIX // 4)
                        xrq = pxr.tile([128, NPIX // 4], F32, tag="xr",
                                       name=f"xr{b}_{c}_{q}", uniquify=True)
                        nc.sync.dma_start(
                            out=xrq, in_=hsv[c * 128:(c + 1) * 128, qsl])
                        if q % 2 == 0:
                            nc.gpsimd.tensor_scalar(
                                out=ot[c][:, qsl], in0=ot[c][:, qsl],
                                scalar1=sc5[c][:, 0:1], scalar2=sc5[c][:, 1:2],
                                op0=ALU.mult, op1=ALU.add)
                        else:
                            nc.vector.tensor_scalar(
                                out=ot[c][:, qsl], in0=ot[c][:, qsl],
                                scalar1=sc5[c][:, 0:1], scalar2=sc5[c][:, 1:2],
                                op0=ALU.mult, op1=ALU.add)
                        nc.vector.tensor_tensor(out=xrq,
                                                in0=ot[c][:, qsl],
                                                in1=xrq, op=ALU.add)
                        nc.sync.dma_start(
                            out=ov[c * 128:(c + 1) * 128, qsl],
                            in_=xrq)

            # ------------------------------------------------ emission
            def scoped(name, fn, *a):
                s, _ = nc.enter_named_scope(name, False)
                fn(*a)
                nc.leave_named_scope(name, s, False)

            scoped("ld_0", ph_load, 0)
            scoped("conv0_0", ph_conv0, 0)
            scoped("ld_1", ph_load, 1)
            scoped("conv1_0", ph_conv1, 0, (0, 1, 2, 3))
            scoped("conv0_1", ph_conv0, 1)
            scoped("attn_0", ph_attn, 0)
            scoped("conv1_1a", ph_conv1, 1, (0,))
            scoped("final_0", ph_final, 0)
            scoped("conv1_1b", ph_conv1, 1, (1, 2, 3))
            scoped("out_0", ph_out, 0)
            scoped("attn_1", ph_attn, 1)
            scoped("final_1", ph_final, 1)
            scoped("out_1", ph_out, 1)

    nc.compile()
    return nc


# ---------------------------------------------------------------- entry

_CACHE = {}


def _get_nc(sim_safe=False):
    key = bool(sim_safe)
    if key not in _CACHE:
        _CACHE[key] = build_nc(sim_safe=key)
    return _CACHE[key]


def make_in_maps(inputs):
    hs_full = np.ascontiguousarray(inputs["hidden_state"], dtype=np.float32)
    wd = _host_weights(
        np.asarray(inputs["w0"], np.float32), np.asarray(inputs["b0"], np.float32),
        np.asarray(inputs["w1"], np.float32), np.asarray(inputs["b1"], np.float32),
        np.asarray(inputs["w2"], np.float32), np.asarray(inputs["b2"], np.float32),
        np.asarray(inputs["w3"], np.float32), np.asarray(inputs["b3"], np.float32),
        np.asarray(inputs["weight"], np.float32))
    cm = _host_consts()
    cpack, bpack = _pack_consts(wd, cm)
    assert cpack.shape[1] == NCF, cpack.shape
    assert bpack.shape[1] == NBF, bpack.shape
    shared = {"cpack": cpack, "bpack": bpack}
    in_maps = []
    for i in range(NCORES):
        m = dict(shared)
        m["hs"] = np.ascontiguousarray(hs_full[i * BPC:(i + 1) * BPC])
        m["hsb"] = m["hs"].astype(ml_dtypes.bfloat16)
        in_maps.append(m)
    return in_maps


def kernel(**inputs):
    from concourse import bass_utils
    nc = _get_nc(sim_safe=False)
    in_maps = make_in_maps(inputs)
    res = bass_utils.run_bass_kernel_spmd(nc, in_maps,
                                          core_ids=list(range(NCORES)))
    out = np.concatenate([res.results[i]["out"] for i in range(NCORES)], axis=0)
    return out.astype(np.float32)
